# revision 18
# baseline (speedup 1.0000x reference)
"""AttnBlock (GroupNorm + single-head attention over HW + residual) on 8 trn2 cores.

Sharding: core = b*4 + qc  (b in 0..1 batch, qc in 0..3 query-column chunk).
Each core receives the full batch element x[b] ([512, 4096], pre-cast bf16)
plus its query chunk x[b][:, qc*1024:(qc+1)*1024] (f32), computes groupnorm +
k/v over all tokens (redundantly per batch) and attention/out-proj for its
1024 query rows.

Layout conventions (everything keyed off channel c = blk*128 + p):
  x/hn, k, q SBUF tiles: [p=128, blk=4, tokens]   (c on partitions)
  vT: [p=128 (token within j-tile), jt=32, c=512] (tokens on partitions)
Attention is computed transposed (S^T[j, i]) so that no on-chip transpose is
ever required: S^T = k(c,j-tile)^T x q(c,i); softmax row-sums accumulate on
DVE over j-tiles and are reduced across partitions with one ones-matmul; the
1/s row is broadcast to 128 partitions with one padded matmul.
"""

import numpy as np
import ml_dtypes

import concourse.bass as bass
import concourse.bacc as bacc
import concourse.mybir as mybir
import concourse.tile as tile
from concourse.bass_utils import run_bass_kernel_spmd

P = 128
C = 512
N = 4096          # tokens per batch element (H*W)
NQ = 1024         # query tokens per core
KB = C // P       # 4 channel blocks
NT = N // 512     # 8 token tiles of 512
JT = N // P       # 32 j tiles of 128
IH = NQ // 512    # 2 query halves of 512
EPS = 1e-6
SCALE = float(C) ** -0.5

F32 = mybir.dt.float32
BF16 = mybir.dt.bfloat16
AF = mybir.ActivationFunctionType
ALU = mybir.AluOpType


def build_nc():
    nc = bacc.Bacc()

    xb = nc.dram_tensor("xb", [C, N], BF16, kind="ExternalInput")
    xq = nc.dram_tensor("xq", [C, NQ], F32, kind="ExternalInput")
    wq = nc.dram_tensor("wq", [C, C], BF16, kind="ExternalInput")  # [cin, cout]
    wk = nc.dram_tensor("wk", [C, C], BF16, kind="ExternalInput")
    wv = nc.dram_tensor("wv", [C, C], BF16, kind="ExternalInput")
    wo = nc.dram_tensor("wo", [C, C], BF16, kind="ExternalInput")
    gcol = nc.dram_tensor("gcol", [P, KB], F32, kind="ExternalInput")   # gamma
    bcol = nc.dram_tensor("bcol", [P, KB], F32, kind="ExternalInput")   # beta
    bkc = nc.dram_tensor("bkc", [P, KB], F32, kind="ExternalInput")     # bk
    bqc = nc.dram_tensor("bqc", [P, KB], F32, kind="ExternalInput")     # bq*SCALE
    boc = nc.dram_tensor("boc", [P, KB], F32, kind="ExternalInput")     # bo
    bv = nc.dram_tensor("bv", [C], F32, kind="ExternalInput")
    gavg = nc.dram_tensor("gavg", [P, P], F32, kind="ExternalInput")
    out = nc.dram_tensor("out", [C, NQ], F32, kind="ExternalOutput")

    xb_r = xb[:].rearrange("(blk p) n -> p blk n", p=P)
    xq_r = xq[:].rearrange("(blk p) n -> p blk n", p=P)
    out_r = out[:].rearrange("(blk p) n -> p blk n", p=P)
    wq_r = wq[:].rearrange("(kb p) co -> p kb co", p=P)
    wk_r = wk[:].rearrange("(kb p) co -> p kb co", p=P)
    wv_r = wv[:].rearrange("(kb p) co -> p kb co", p=P)
    wo_r = wo[:].rearrange("(kb p) co -> p kb co", p=P)

    with tile.TileContext(nc) as tc:
        with (
            tc.tile_pool(name="big", bufs=1) as big,
            tc.tile_pool(name="st", bufs=1) as st,
            tc.tile_pool(name="et", bufs=8) as etp,
            tc.tile_pool(name="ep", bufs=2) as ep,
            tc.tile_pool(name="epo", bufs=4) as epo,
            tc.tile_pool(name="mm", bufs=3, space="PSUM") as psmm,
            tc.tile_pool(name="pvp", bufs=1, space="PSUM") as pvp,
        ):
            # ---- persistent tiles ----
            x_sb = big.tile([P, KB, N], BF16)    # x, normalized in place -> hn
            hq = big.tile([P, KB, NQ], BF16)
            k_sb = big.tile([P, KB, N], BF16)
            vT = big.tile([P, JT, C], BF16)
            q_sb = big.tile([P, KB, NQ], BF16)
            xq_sb = big.tile([P, KB, NQ], F32)
            wq_sb = big.tile([P, KB, C], BF16)
            wk_sb = big.tile([P, KB, C], BF16)
            wv_sb = big.tile([P, KB, C], BF16)
            wo_sb = big.tile([P, KB, C], BF16)
            gcol_sb = big.tile([P, KB], F32)
            bcol_sb = big.tile([P, KB], F32)
            bkc_sb = big.tile([P, KB], F32)
            bqc_sb = big.tile([P, KB], F32)
            boc_sb = big.tile([P, KB], F32)
            bv_sb = big.tile([P, 512], F32)
            gavg_sb = big.tile([P, P], F32)
            ones_f = big.tile([P, 1], F32)
            e0 = big.tile([P, P], F32)      # row 0 = 1, else 0 (for row bcast)
            rpad = big.tile([P, 512], F32)  # row 0 = 1/s, else 0

            # x streams in first so bn_stats can start ASAP
            XPIECES = [(nt * 512, 512) for nt in range(NT)]
            for off, w in XPIECES:
                nc.sync.dma_start(out=x_sb[:, :, off:off + w],
                                  in_=xb_r[:, :, off:off + w])
            nc.sync.dma_start(out=wq_sb, in_=wq_r)
            nc.sync.dma_start(out=wk_sb, in_=wk_r)
            nc.sync.dma_start(out=wv_sb, in_=wv_r)
            nc.sync.dma_start(out=wo_sb, in_=wo_r)
            nc.sync.dma_start(out=gcol_sb, in_=gcol[:])
            nc.sync.dma_start(out=bcol_sb, in_=bcol[:])
            nc.sync.dma_start(out=bkc_sb, in_=bkc[:])
            nc.sync.dma_start(out=bqc_sb, in_=bqc[:])
            nc.sync.dma_start(out=boc_sb, in_=boc[:])
            nc.sync.dma_start(out=gavg_sb, in_=gavg[:])
            # broadcast bv across all partitions (partition step 0)
            bv_ap = bv[:]
            nc.gpsimd.dma_start(
                out=bv_sb,
                in_=bass.AP(tensor=bv_ap.tensor, offset=bv_ap.offset,
                            ap=[[0, P], *bv_ap.ap]),
            )
            nc.vector.memset(ones_f, 1.0)
            nc.vector.memset(e0, 0.0)
            nc.vector.memset(e0[0:1, :], 1.0)
            nc.vector.memset(rpad, 0.0)
            nc.sync.dma_start(out=xq_sb, in_=xq_r)

            # ---- stage A: groupnorm stats ----
            NPC = len(XPIECES)
            stats = st.tile([P, KB, NPC, 6], F32)
            for i, (off, w) in enumerate(XPIECES):
                for kb in range(KB):
                    nc.vector.bn_stats(out=stats[:, kb, i, :],
                                       in_=x_sb[:, kb, off:off + w])
            mv = st.tile([P, KB, 2], F32)
            for kb in range(KB):
                nc.vector.bn_aggr(out=mv[:, kb, :], in_=stats[:, kb, :, :])

            # per-partition stats -> per-group stats -> per-channel A/D
            stat8 = st.tile([P, 8], F32)
            nc.vector.tensor_copy(out=stat8[:, 0:4], in_=mv[:, :, 0])
            nc.vector.tensor_tensor(out=stat8[:, 4:8], in0=mv[:, :, 0],
                                    in1=mv[:, :, 0], op=ALU.mult)
            nc.vector.tensor_tensor(out=stat8[:, 4:8], in0=stat8[:, 4:8],
                                    in1=mv[:, :, 1], op=ALU.add)
            psb = psmm.tile([P, 8], F32, tag="mm", name="psb")
            nc.tensor.matmul(psb, gavg_sb, stat8, start=True, stop=True)
            mq = st.tile([P, 8], F32)
            nc.vector.tensor_copy(out=mq, in_=psb)
            varg = st.tile([P, 4], F32)
            nc.vector.tensor_tensor(out=varg, in0=mq[:, 0:4], in1=mq[:, 0:4],
                                    op=ALU.mult)
            nc.vector.tensor_tensor(out=varg, in0=mq[:, 4:8], in1=varg,
                                    op=ALU.subtract)
            rstd = st.tile([P, 4], F32)
            eps_sb = st.tile([P, 1], F32)
            nc.vector.memset(eps_sb, EPS)
            nc.scalar.activation(out=rstd, in_=varg, func=AF.Sqrt, bias=eps_sb)
            nc.vector.reciprocal(out=rstd, in_=rstd)
            expdump = st.tile([P, 1], F32)
            nc.scalar.activation(out=expdump, in_=eps_sb, func=AF.Exp)
            # A = rstd*gamma ; D = beta - mean*A  (hn = A*x + D)
            A = st.tile([P, 4], F32)
            D = st.tile([P, 4], F32)
            nc.vector.tensor_tensor(out=A, in0=rstd, in1=gcol_sb, op=ALU.mult)
            nc.vector.tensor_tensor(out=D, in0=mq[:, 0:4], in1=A, op=ALU.mult)
            nc.vector.tensor_tensor(out=D, in0=bcol_sb, in1=D, op=ALU.subtract)
            # ---- stage A/B interleaved: normalize x in place chunk-wise and
            # run k/vT projections on each chunk as soon as it is ready.
            for nt in range(NT):
                for kb in range(KB):
                    sl = x_sb[:, kb, nt * 512:(nt + 1) * 512]
                    nc.vector.tensor_scalar(
                        out=sl, in0=sl,
                        scalar1=A[:, kb:kb + 1], scalar2=D[:, kb:kb + 1],
                        op0=ALU.mult, op1=ALU.add)
                # k projection for this chunk
                for blk in range(KB):
                    pk = psmm.tile([P, 512], F32, tag="mm", name="pk")
                    for kb in range(KB):
                        nc.tensor.matmul(
                            pk, wk_sb[:, kb, blk * P:(blk + 1) * P],
                            x_sb[:, kb, nt * 512:(nt + 1) * 512],
                            start=(kb == 0), stop=(kb == KB - 1))
                    nc.vector.tensor_scalar_add(
                        k_sb[:, blk, nt * 512:(nt + 1) * 512], pk,
                        bkc_sb[:, blk:blk + 1])
                # vT for this chunk's 4 j-tiles
                for j4 in range(4):
                    jt = nt * 4 + j4
                    pv = psmm.tile([P, 512], F32, tag="mm", name="pv")
                    for kb in range(KB):
                        nc.tensor.matmul(
                            pv, x_sb[:, kb, jt * P:(jt + 1) * P], wv_sb[:, kb, :],
                            start=(kb == 0), stop=(kb == KB - 1))
                    nc.vector.tensor_tensor(out=vT[:, jt, :], in0=pv, in1=bv_sb,
                                            op=ALU.add)

            # hq here: off the stats -> first-k-projection critical path
            Aq = st.tile([P, 4], F32)
            Dq = st.tile([P, 4], F32)
            nc.vector.tensor_scalar_mul(Aq, A, SCALE)
            nc.vector.tensor_scalar_mul(Dq, D, SCALE)
            for kb in range(KB):
                nc.vector.tensor_scalar(
                    out=hq[:, kb, :], in0=xq_sb[:, kb, :],
                    scalar1=Aq[:, kb:kb + 1], scalar2=Dq[:, kb:kb + 1],
                    op0=ALU.mult, op1=ALU.add)
            # fold the out-proj bias into the residual: xq_sb += bo (per chan)
            for kb in range(KB):
                nc.vector.tensor_scalar_add(
                    xq_sb[:, kb, :], xq_sb[:, kb, :], boc_sb[:, kb:kb + 1])
            # q = (Wq @ hq_scaled) + bq*SCALE  (SCALE folded into hq/bqc)
            for blk in range(KB):
                for i2 in range(IH):
                    pq = psmm.tile([P, 512], F32, tag="mm", name="pq")
                    for kb in range(KB):
                        nc.tensor.matmul(
                            pq, wq_sb[:, kb, blk * P:(blk + 1) * P],
                            hq[:, kb, i2 * 512:(i2 + 1) * 512],
                            start=(kb == 0), stop=(kb == KB - 1))
                    nc.vector.tensor_scalar_add(
                        q_sb[:, blk, i2 * 512:(i2 + 1) * 512], pq,
                        bqc_sb[:, blk:blk + 1])

            # ---- stage C: attention, pipelined over (ih, jt) ----
            # Per step: scores+exp for (ih, jt); PV matmuls for the previous
            # step; DVE row-sum accumulation into sden_sb. The ih=0 epilogue
            # is emitted a few steps into ih=1 so PE never waits on the
            # reciprocal/broadcast chain.
            pv_ps = {}
            sden_sb = {}
            ets = {}

            def epilogue(ih):
                # Copy unnormalized PV to SBUF right away (frees the pv psum
                # accumulators for the next half) and run the Wo projection on
                # it; the softmax 1/s is folded in afterwards:
                #   out = (Wo @ PV)/s + bo + xq
                on = ep.tile([P, KB, 512], BF16, tag="on", name=f"on{ih}")
                for cc in range(KB):
                    nc.vector.tensor_copy(out=on[:, cc, :], in_=pv_ps[ih][cc])
                # cross-partition sum of sden_sb -> s[1, 512]; r = 1/s
                sden = pvp.tile([1, 512], F32, tag="sden", name=f"sden{ih}")
                nc.tensor.matmul(sden, ones_f, sden_sb[ih], start=True, stop=True)
                nc.vector.reciprocal(out=rpad[0:1, :], in_=sden)
                rb = psmm.tile([P, 512], F32, tag="mm", name=f"rb{ih}")
                nc.tensor.matmul(rb, e0, rpad, start=True, stop=True)
                rbs = ep.tile([P, 512], F32, tag="rbs", name=f"rbs{ih}")
                nc.vector.tensor_copy(out=rbs, in_=rb)
                for blk in range(KB):
                    po = psmm.tile([P, 512], F32, tag="mm", name=f"po{ih}")
                    for cc in range(KB):
                        nc.tensor.matmul(
                            po, wo_sb[:, cc, blk * P:(blk + 1) * P], on[:, cc, :],
                            start=(cc == 0), stop=(cc == KB - 1))
                    ot = epo.tile([P, 512], F32, tag="ot", name=f"ot{ih}")
                    nc.vector.tensor_tensor(out=ot, in0=po, in1=rbs, op=ALU.mult)
                    nc.vector.tensor_tensor(
                        out=ot, in0=ot,
                        in1=xq_sb[:, blk, ih * 512:(ih + 1) * 512], op=ALU.add)
                    nc.sync.dma_start(
                        out=out_r[:, blk, ih * 512:(ih + 1) * 512], in_=ot)

            NSTEP = IH * JT
            for step in range(NSTEP + 1):
                if step < NSTEP:
                    ih, jt = divmod(step, JT)
                    if jt == 0:
                        pv_ps[ih] = [pvp.tile([P, 512], F32, tag=f"pv{cc}",
                                              name=f"pv{ih}_{cc}")
                                     for cc in range(KB)]
                        sden_sb[ih] = ep.tile([P, 512], F32, tag="sd",
                                              name=f"sd{ih}")
                    ss = psmm.tile([P, 512], F32, tag="mm", name="ss")
                    for kb in range(KB):
                        nc.tensor.matmul(
                            ss, k_sb[:, kb, jt * P:(jt + 1) * P],
                            q_sb[:, kb, ih * 512:(ih + 1) * 512],
                            start=(kb == 0), stop=(kb == KB - 1))
                    et = etp.tile([P, 512], BF16, tag="et", name="et")
                    nc.scalar.activation(out=et, in_=ss, func=AF.Exp)
                    ets[step] = et
                    if jt == 0:
                        nc.vector.tensor_copy(out=sden_sb[ih], in_=et)
                    else:
                        nc.vector.tensor_tensor(out=sden_sb[ih], in0=sden_sb[ih],
                                                in1=et, op=ALU.add)
                if step >= 1:
                    pih, pjt = divmod(step - 1, JT)
                    et = ets.pop(step - 1)
                    for cc in range(KB):
                        nc.tensor.matmul(
                            pv_ps[pih][cc], vT[:, pjt, cc * P:(cc + 1) * P],
                            et, start=(pjt == 0), stop=(pjt == JT - 1))
                if step == JT + 2:
                    epilogue(0)
            epilogue(1)

    nc.finalize()
    return nc


_NC = None


def _get_nc():
    global _NC
    if _NC is None:
        _NC = build_nc()
    return _NC


def _col(v):
    """[C] f32 -> [P, KB] with c = blk*128 + p."""
    return np.ascontiguousarray(np.asarray(v, np.float32).reshape(KB, P).T)


def _make_in_maps(inputs):
    x = np.asarray(inputs["x"], np.float32).reshape(2, C, N)
    x_bf = x.astype(ml_dtypes.bfloat16)
    wqT = np.ascontiguousarray(np.asarray(inputs["Wq"], np.float32).T).astype(ml_dtypes.bfloat16)
    wkT = np.ascontiguousarray(np.asarray(inputs["Wk"], np.float32).T).astype(ml_dtypes.bfloat16)
    wvT = np.ascontiguousarray(np.asarray(inputs["Wv"], np.float32).T).astype(ml_dtypes.bfloat16)
    woT = np.ascontiguousarray(np.asarray(inputs["Wo"], np.float32).T).astype(ml_dtypes.bfloat16)
    gcol = _col(inputs["gamma"])
    bcol = _col(inputs["beta"])
    bkc = _col(inputs["bk"])
    bqc = _col(np.asarray(inputs["bq"], np.float32) * SCALE)
    boc = _col(inputs["bo"])
    bvv = np.ascontiguousarray(np.asarray(inputs["bv"], np.float32))

    pidx = np.arange(P)
    gavg = np.where(pidx[:, None] // 16 == pidx[None, :] // 16,
                    np.float32(1.0 / 16.0), np.float32(0.0))

    common = dict(wq=wqT, wk=wkT, wv=wvT, wo=woT, gcol=gcol, bcol=bcol,
                  bkc=bkc, bqc=bqc, boc=boc, bv=bvv, gavg=gavg)
    in_maps = []
    for core in range(8):
        b, qc = core // 4, core % 4
        in_maps.append(dict(
            common,
            xb=np.ascontiguousarray(x_bf[b]),
            xq=np.ascontiguousarray(x[b][:, qc * NQ:(qc + 1) * NQ]),
        ))
    return in_maps


def run(inputs, trace=False):
    nc = _get_nc()
    in_maps = _make_in_maps(inputs)
    res = run_bass_kernel_spmd(nc, in_maps, core_ids=list(range(8)), trace=trace)
    y = np.empty((2, C, N), np.float32)
    for core in range(8):
        b, qc = core // 4, core % 4
        y[b][:, qc * NQ:(qc + 1) * NQ] = res.results[core]["out"]
    return y.reshape(2, C, 64, 64), res


def kernel(**inputs):
    y, _ = run(inputs, trace=False)
    return y


# revision 21
# speedup vs baseline: 1.0047x; 1.0047x over previous
"""AttnBlock (GroupNorm + single-head attention over HW + residual) on 8 trn2 cores.

Sharding: core = b*4 + qc  (b in 0..1 batch, qc in 0..3 query-column chunk).
Each core receives the full batch element x[b] ([512, 4096], pre-cast bf16)
plus its query chunk x[b][:, qc*1024:(qc+1)*1024] (f32), computes groupnorm +
k/v over all tokens (redundantly per batch) and attention/out-proj for its
1024 query rows.

Layout conventions (everything keyed off channel c = blk*128 + p):
  x/hn, k, q SBUF tiles: [p=128, blk=4, tokens]   (c on partitions)
  vT: [p=128 (token within j-tile), jt=32, c=512] (tokens on partitions)
Attention is computed transposed (S^T[j, i]) so that no on-chip transpose is
ever required: S^T = k(c,j-tile)^T x q(c,i); softmax row-sums accumulate on
DVE over j-tiles and are reduced across partitions with one ones-matmul; the
1/s row is broadcast to 128 partitions with one padded matmul.
"""

import numpy as np
import ml_dtypes

import concourse.bass as bass
import concourse.bacc as bacc
import concourse.mybir as mybir
import concourse.tile as tile
from concourse.bass_utils import run_bass_kernel_spmd

P = 128
C = 512
N = 4096          # tokens per batch element (H*W)
NQ = 1024         # query tokens per core
KB = C // P       # 4 channel blocks
NT = N // 512     # 8 token tiles of 512
JT = N // P       # 32 j tiles of 128
IH = NQ // 512    # 2 query halves of 512
EPS = 1e-6
SCALE = float(C) ** -0.5

F32 = mybir.dt.float32
BF16 = mybir.dt.bfloat16
AF = mybir.ActivationFunctionType
ALU = mybir.AluOpType


def build_nc():
    nc = bacc.Bacc()

    xb = nc.dram_tensor("xb", [C, N], BF16, kind="ExternalInput")
    xq = nc.dram_tensor("xq", [C, NQ], F32, kind="ExternalInput")
    wq = nc.dram_tensor("wq", [C, C], BF16, kind="ExternalInput")  # [cin, cout]
    wk = nc.dram_tensor("wk", [C, C], BF16, kind="ExternalInput")
    wv = nc.dram_tensor("wv", [C, C], BF16, kind="ExternalInput")
    wo = nc.dram_tensor("wo", [C, C], BF16, kind="ExternalInput")
    gcol = nc.dram_tensor("gcol", [P, KB], F32, kind="ExternalInput")   # gamma
    bcol = nc.dram_tensor("bcol", [P, KB], F32, kind="ExternalInput")   # beta
    bkc = nc.dram_tensor("bkc", [P, KB], F32, kind="ExternalInput")     # bk
    bqc = nc.dram_tensor("bqc", [P, KB], F32, kind="ExternalInput")     # bq*SCALE
    boc = nc.dram_tensor("boc", [P, KB], F32, kind="ExternalInput")     # bo
    bv = nc.dram_tensor("bv", [C], F32, kind="ExternalInput")
    gavg = nc.dram_tensor("gavg", [P, P], F32, kind="ExternalInput")
    out = nc.dram_tensor("out", [C, NQ], F32, kind="ExternalOutput")

    xb_r = xb[:].rearrange("(blk p) n -> p blk n", p=P)
    xq_r = xq[:].rearrange("(blk p) n -> p blk n", p=P)
    out_r = out[:].rearrange("(blk p) n -> p blk n", p=P)
    wq_r = wq[:].rearrange("(kb p) co -> p kb co", p=P)
    wk_r = wk[:].rearrange("(kb p) co -> p kb co", p=P)
    wv_r = wv[:].rearrange("(kb p) co -> p kb co", p=P)
    wo_r = wo[:].rearrange("(kb p) co -> p kb co", p=P)

    with tile.TileContext(nc) as tc:
        with (
            tc.tile_pool(name="big", bufs=1) as big,
            tc.tile_pool(name="st", bufs=1) as st,
            tc.tile_pool(name="et", bufs=8) as etp,
            tc.tile_pool(name="ep", bufs=2) as ep,
            tc.tile_pool(name="epo", bufs=4) as epo,
            tc.tile_pool(name="mm", bufs=3, space="PSUM") as psmm,
            tc.tile_pool(name="pvp", bufs=1, space="PSUM") as pvp,
        ):
            # ---- persistent tiles ----
            x_sb = big.tile([P, KB, N], BF16)    # x, normalized in place -> hn
            hq = big.tile([P, KB, NQ], BF16)
            k_sb = big.tile([P, KB, N], BF16)
            vT = big.tile([P, JT, C], BF16)
            q_sb = big.tile([P, KB, NQ], BF16)
            xq_sb = big.tile([P, KB, NQ], F32)
            wq_sb = big.tile([P, KB, C], BF16)
            wk_sb = big.tile([P, KB, C], BF16)
            wv_sb = big.tile([P, KB, C], BF16)
            wo_sb = big.tile([P, KB, C], BF16)
            gcol_sb = big.tile([P, KB], F32)
            bcol_sb = big.tile([P, KB], F32)
            bkc_sb = big.tile([P, KB], F32)
            bqc_sb = big.tile([P, KB], F32)
            boc_sb = big.tile([P, KB], F32)
            bv_sb = big.tile([P, 512], F32)
            gavg_sb = big.tile([P, P], F32)
            ones_f = big.tile([P, 1], F32)
            e0 = big.tile([P, P], F32)      # row 0 = 1, else 0 (for row bcast)
            rpad = big.tile([P, 512], F32)  # row 0 = 1/s, else 0

            # x streams in first so bn_stats can start ASAP
            XPIECES = [(nt * 512, 512) for nt in range(NT)]
            for off, w in XPIECES:
                nc.sync.dma_start(out=x_sb[:, :, off:off + w],
                                  in_=xb_r[:, :, off:off + w])
            nc.sync.dma_start(out=wq_sb, in_=wq_r)
            nc.sync.dma_start(out=wk_sb, in_=wk_r)
            nc.sync.dma_start(out=wv_sb, in_=wv_r)
            nc.sync.dma_start(out=wo_sb, in_=wo_r)
            nc.sync.dma_start(out=gcol_sb, in_=gcol[:])
            nc.sync.dma_start(out=bcol_sb, in_=bcol[:])
            nc.sync.dma_start(out=bkc_sb, in_=bkc[:])
            nc.sync.dma_start(out=bqc_sb, in_=bqc[:])
            nc.sync.dma_start(out=boc_sb, in_=boc[:])
            nc.sync.dma_start(out=gavg_sb, in_=gavg[:])
            # broadcast bv across all partitions (partition step 0)
            bv_ap = bv[:]
            nc.gpsimd.dma_start(
                out=bv_sb,
                in_=bass.AP(tensor=bv_ap.tensor, offset=bv_ap.offset,
                            ap=[[0, P], *bv_ap.ap]),
            )
            nc.vector.memset(ones_f, 1.0)
            nc.vector.memset(e0, 0.0)
            nc.vector.memset(e0[0:1, :], 1.0)
            nc.vector.memset(rpad, 0.0)
            nc.sync.dma_start(out=xq_sb, in_=xq_r)

            # ---- stage A: groupnorm stats ----
            # Split across engines: channel-block 0 of each chunk goes to ACT
            # (Copy/Square passes with accum_out -> per-partition sums),
            # blocks 1..3 go to DVE bn_stats. Both finish ~5us sooner than
            # DVE-alone. ACT passes are grouped by func to avoid table loads.
            NPC = len(XPIECES)
            stats = st.tile([P, KB - 1, NPC, 6], F32)
            adump = st.tile([P, 512], F32)
            accs = st.tile([P, NPC], F32)
            accq = st.tile([P, NPC], F32)
            for i, (off, w) in enumerate(XPIECES):
                nc.scalar.activation(out=adump[:, :w], in_=x_sb[:, 0, off:off + w],
                                     func=AF.Copy, accum_out=accs[:, i:i + 1])
                for kb in range(1, KB):
                    nc.vector.bn_stats(out=stats[:, kb - 1, i, :],
                                       in_=x_sb[:, kb, off:off + w])
            for i, (off, w) in enumerate(XPIECES):
                nc.scalar.activation(out=adump[:, :w], in_=x_sb[:, 0, off:off + w],
                                     func=AF.Square, accum_out=accq[:, i:i + 1])
            mv = st.tile([P, KB - 1, 2], F32)
            for kb in range(1, KB):
                nc.vector.bn_aggr(out=mv[:, kb - 1, :], in_=stats[:, kb - 1, :, :])

            # per-partition stats -> per-group stats -> per-channel A/D
            stat8 = st.tile([P, 8], F32)
            sm0 = st.tile([P, 2], F32)
            nc.vector.reduce_sum(out=sm0[:, 0:1], in_=accs,
                                 axis=mybir.AxisListType.X)
            nc.vector.reduce_sum(out=sm0[:, 1:2], in_=accq,
                                 axis=mybir.AxisListType.X)
            nc.vector.tensor_scalar_mul(sm0, sm0, 1.0 / float(N))
            nc.vector.tensor_copy(out=stat8[:, 0:1], in_=sm0[:, 0:1])
            nc.vector.tensor_copy(out=stat8[:, 4:5], in_=sm0[:, 1:2])
            nc.vector.tensor_copy(out=stat8[:, 1:4], in_=mv[:, :, 0])
            nc.vector.tensor_tensor(out=stat8[:, 5:8], in0=mv[:, :, 0],
                                    in1=mv[:, :, 0], op=ALU.mult)
            nc.vector.tensor_tensor(out=stat8[:, 5:8], in0=stat8[:, 5:8],
                                    in1=mv[:, :, 1], op=ALU.add)
            psb = psmm.tile([P, 8], F32, tag="mm", name="psb")
            nc.tensor.matmul(psb, gavg_sb, stat8, start=True, stop=True)
            mq = st.tile([P, 8], F32)
            nc.vector.tensor_copy(out=mq, in_=psb)
            varg = st.tile([P, 4], F32)
            nc.vector.tensor_tensor(out=varg, in0=mq[:, 0:4], in1=mq[:, 0:4],
                                    op=ALU.mult)
            nc.vector.tensor_tensor(out=varg, in0=mq[:, 4:8], in1=varg,
                                    op=ALU.subtract)
            rstd = st.tile([P, 4], F32)
            eps_sb = st.tile([P, 1], F32)
            nc.vector.memset(eps_sb, EPS)
            nc.scalar.activation(out=rstd, in_=varg, func=AF.Sqrt, bias=eps_sb)
            nc.vector.reciprocal(out=rstd, in_=rstd)
            expdump = st.tile([P, 1], F32)
            nc.scalar.activation(out=expdump, in_=eps_sb, func=AF.Exp)
            # A = rstd*gamma ; D = beta - mean*A  (hn = A*x + D)
            A = st.tile([P, 4], F32)
            D = st.tile([P, 4], F32)
            nc.vector.tensor_tensor(out=A, in0=rstd, in1=gcol_sb, op=ALU.mult)
            nc.vector.tensor_tensor(out=D, in0=mq[:, 0:4], in1=A, op=ALU.mult)
            nc.vector.tensor_tensor(out=D, in0=bcol_sb, in1=D, op=ALU.subtract)
            # ---- stage A/B interleaved: normalize x in place chunk-wise and
            # run k/vT projections on each chunk as soon as it is ready.
            for nt in range(NT):
                for kb in range(KB):
                    sl = x_sb[:, kb, nt * 512:(nt + 1) * 512]
                    nc.vector.tensor_scalar(
                        out=sl, in0=sl,
                        scalar1=A[:, kb:kb + 1], scalar2=D[:, kb:kb + 1],
                        op0=ALU.mult, op1=ALU.add)
                # k projection for this chunk
                for blk in range(KB):
                    pk = psmm.tile([P, 512], F32, tag="mm", name="pk")
                    for kb in range(KB):
                        nc.tensor.matmul(
                            pk, wk_sb[:, kb, blk * P:(blk + 1) * P],
                            x_sb[:, kb, nt * 512:(nt + 1) * 512],
                            start=(kb == 0), stop=(kb == KB - 1))
                    nc.vector.tensor_scalar_add(
                        k_sb[:, blk, nt * 512:(nt + 1) * 512], pk,
                        bkc_sb[:, blk:blk + 1])
                # vT for this chunk's 4 j-tiles
                for j4 in range(4):
                    jt = nt * 4 + j4
                    pv = psmm.tile([P, 512], F32, tag="mm", name="pv")
                    for kb in range(KB):
                        nc.tensor.matmul(
                            pv, x_sb[:, kb, jt * P:(jt + 1) * P], wv_sb[:, kb, :],
                            start=(kb == 0), stop=(kb == KB - 1))
                    nc.vector.tensor_tensor(out=vT[:, jt, :], in0=pv, in1=bv_sb,
                                            op=ALU.add)

            # hq here: off the stats -> first-k-projection critical path
            Aq = st.tile([P, 4], F32)
            Dq = st.tile([P, 4], F32)
            nc.vector.tensor_scalar_mul(Aq, A, SCALE)
            nc.vector.tensor_scalar_mul(Dq, D, SCALE)
            for kb in range(KB):
                nc.vector.tensor_scalar(
                    out=hq[:, kb, :], in0=xq_sb[:, kb, :],
                    scalar1=Aq[:, kb:kb + 1], scalar2=Dq[:, kb:kb + 1],
                    op0=ALU.mult, op1=ALU.add)
            # fold the out-proj bias into the residual: xq_sb += bo (per chan)
            for kb in range(KB):
                nc.vector.tensor_scalar_add(
                    xq_sb[:, kb, :], xq_sb[:, kb, :], boc_sb[:, kb:kb + 1])
            # q = (Wq @ hq_scaled) + bq*SCALE  (SCALE folded into hq/bqc)
            for blk in range(KB):
                for i2 in range(IH):
                    pq = psmm.tile([P, 512], F32, tag="mm", name="pq")
                    for kb in range(KB):
                        nc.tensor.matmul(
                            pq, wq_sb[:, kb, blk * P:(blk + 1) * P],
                            hq[:, kb, i2 * 512:(i2 + 1) * 512],
                            start=(kb == 0), stop=(kb == KB - 1))
                    nc.vector.tensor_scalar_add(
                        q_sb[:, blk, i2 * 512:(i2 + 1) * 512], pq,
                        bqc_sb[:, blk:blk + 1])

            # ---- stage C: attention, pipelined over (ih, jt) ----
            # Per step: scores+exp for (ih, jt); PV matmuls for the previous
            # step; DVE row-sum accumulation into sden_sb. The ih=0 epilogue
            # is emitted a few steps into ih=1 so PE never waits on the
            # reciprocal/broadcast chain.
            pv_ps = {}
            sden_sb = {}
            ets = {}

            def epilogue(ih):
                # Copy unnormalized PV to SBUF right away (frees the pv psum
                # accumulators for the next half) and run the Wo projection on
                # it; the softmax 1/s is folded in afterwards:
                #   out = (Wo @ PV)/s + bo + xq
                on = ep.tile([P, KB, 512], BF16, tag="on", name=f"on{ih}")
                for cc in range(KB):
                    nc.vector.tensor_copy(out=on[:, cc, :], in_=pv_ps[ih][cc])
                # cross-partition sum of sden_sb -> s[1, 512]; r = 1/s
                sden = pvp.tile([1, 512], F32, tag="sden", name=f"sden{ih}")
                nc.tensor.matmul(sden, ones_f, sden_sb[ih], start=True, stop=True)
                nc.vector.reciprocal(out=rpad[0:1, :], in_=sden)
                rb = psmm.tile([P, 512], F32, tag="mm", name=f"rb{ih}")
                nc.tensor.matmul(rb, e0, rpad, start=True, stop=True)
                rbs = ep.tile([P, 512], F32, tag="rbs", name=f"rbs{ih}")
                nc.vector.tensor_copy(out=rbs, in_=rb)
                for blk in range(KB):
                    po = psmm.tile([P, 512], F32, tag="mm", name=f"po{ih}")
                    for cc in range(KB):
                        nc.tensor.matmul(
                            po, wo_sb[:, cc, blk * P:(blk + 1) * P], on[:, cc, :],
                            start=(cc == 0), stop=(cc == KB - 1))
                    ot = epo.tile([P, 512], F32, tag="ot", name=f"ot{ih}")
                    nc.vector.tensor_tensor(out=ot, in0=po, in1=rbs, op=ALU.mult)
                    nc.vector.tensor_tensor(
                        out=ot, in0=ot,
                        in1=xq_sb[:, blk, ih * 512:(ih + 1) * 512], op=ALU.add)
                    nc.sync.dma_start(
                        out=out_r[:, blk, ih * 512:(ih + 1) * 512], in_=ot)

            NSTEP = IH * JT
            for step in range(NSTEP + 1):
                if step < NSTEP:
                    ih, jt = divmod(step, JT)
                    if jt == 0:
                        pv_ps[ih] = [pvp.tile([P, 512], F32, tag=f"pv{cc}",
                                              name=f"pv{ih}_{cc}")
                                     for cc in range(KB)]
                        sden_sb[ih] = ep.tile([P, 512], F32, tag="sd",
                                              name=f"sd{ih}")
                    ss = psmm.tile([P, 512], F32, tag="mm", name="ss")
                    for kb in range(KB):
                        nc.tensor.matmul(
                            ss, k_sb[:, kb, jt * P:(jt + 1) * P],
                            q_sb[:, kb, ih * 512:(ih + 1) * 512],
                            start=(kb == 0), stop=(kb == KB - 1))
                    et = etp.tile([P, 512], BF16, tag="et", name="et")
                    nc.scalar.activation(out=et, in_=ss, func=AF.Exp)
                    ets[step] = et
                    if jt == 0:
                        nc.vector.tensor_copy(out=sden_sb[ih], in_=et)
                    else:
                        nc.vector.tensor_tensor(out=sden_sb[ih], in0=sden_sb[ih],
                                                in1=et, op=ALU.add)
                if step >= 1:
                    pih, pjt = divmod(step - 1, JT)
                    et = ets.pop(step - 1)
                    for cc in range(KB):
                        nc.tensor.matmul(
                            pv_ps[pih][cc], vT[:, pjt, cc * P:(cc + 1) * P],
                            et, start=(pjt == 0), stop=(pjt == JT - 1))
                if step == JT + 2:
                    epilogue(0)
            epilogue(1)

    nc.finalize()
    return nc


_NC = None


def _get_nc():
    global _NC
    if _NC is None:
        _NC = build_nc()
    return _NC


def _col(v):
    """[C] f32 -> [P, KB] with c = blk*128 + p."""
    return np.ascontiguousarray(np.asarray(v, np.float32).reshape(KB, P).T)


def _make_in_maps(inputs):
    x = np.asarray(inputs["x"], np.float32).reshape(2, C, N)
    x_bf = x.astype(ml_dtypes.bfloat16)
    wqT = np.ascontiguousarray(np.asarray(inputs["Wq"], np.float32).T).astype(ml_dtypes.bfloat16)
    wkT = np.ascontiguousarray(np.asarray(inputs["Wk"], np.float32).T).astype(ml_dtypes.bfloat16)
    wvT = np.ascontiguousarray(np.asarray(inputs["Wv"], np.float32).T).astype(ml_dtypes.bfloat16)
    woT = np.ascontiguousarray(np.asarray(inputs["Wo"], np.float32).T).astype(ml_dtypes.bfloat16)
    gcol = _col(inputs["gamma"])
    bcol = _col(inputs["beta"])
    bkc = _col(inputs["bk"])
    bqc = _col(np.asarray(inputs["bq"], np.float32) * SCALE)
    boc = _col(inputs["bo"])
    bvv = np.ascontiguousarray(np.asarray(inputs["bv"], np.float32))

    pidx = np.arange(P)
    gavg = np.where(pidx[:, None] // 16 == pidx[None, :] // 16,
                    np.float32(1.0 / 16.0), np.float32(0.0))

    common = dict(wq=wqT, wk=wkT, wv=wvT, wo=woT, gcol=gcol, bcol=bcol,
                  bkc=bkc, bqc=bqc, boc=boc, bv=bvv, gavg=gavg)
    in_maps = []
    for core in range(8):
        b, qc = core // 4, core % 4
        in_maps.append(dict(
            common,
            xb=np.ascontiguousarray(x_bf[b]),
            xq=np.ascontiguousarray(x[b][:, qc * NQ:(qc + 1) * NQ]),
        ))
    return in_maps


def run(inputs, trace=False):
    nc = _get_nc()
    in_maps = _make_in_maps(inputs)
    res = run_bass_kernel_spmd(nc, in_maps, core_ids=list(range(8)), trace=trace)
    y = np.empty((2, C, N), np.float32)
    for core in range(8):
        b, qc = core // 4, core % 4
        y[b][:, qc * NQ:(qc + 1) * NQ] = res.results[core]["out"]
    return y.reshape(2, C, 64, 64), res


def kernel(**inputs):
    y, _ = run(inputs, trace=False)
    return y


# revision 28
# speedup vs baseline: 1.0954x; 1.0902x over previous
"""AttnBlock (GroupNorm + single-head attention over HW + residual) on 8 trn2 cores.

Sharding: core = b*4 + qc  (b in 0..1 batch, qc in 0..3 query-column chunk).
Each core receives the full batch element x[b] ([512, 4096], pre-cast bf16)
plus its query chunk x[b][:, qc*1024:(qc+1)*1024] (f32), computes groupnorm +
k/v over all tokens (redundantly per batch) and attention/out-proj for its
1024 query rows.

Layout conventions (everything keyed off channel c = blk*128 + p):
  x/hn, k, q SBUF tiles: [p=128, blk=4, tokens]   (c on partitions)
  vT: [p=128 (token within j-tile), jt=32, c=512] (tokens on partitions)
Attention is computed transposed (S^T[j, i]) so that no on-chip transpose is
ever required: S^T = k(c,j-tile)^T x q(c,i); softmax row-sums accumulate on
DVE over j-tiles and are reduced across partitions with one ones-matmul; the
1/s row is broadcast to 128 partitions with one padded matmul.
"""

import numpy as np
import ml_dtypes

import concourse.bass as bass
import concourse.bacc as bacc
import concourse.mybir as mybir
import concourse.tile as tile
from concourse.bass_utils import run_bass_kernel_spmd

P = 128
C = 512
N = 4096          # tokens per batch element (H*W)
NQ = 1024         # query tokens per core
KB = C // P       # 4 channel blocks
NT = N // 512     # 8 token tiles of 512
JT = N // P       # 32 j tiles of 128
IH = NQ // 512    # 2 query halves of 512
EPS = 1e-6
SCALE = float(C) ** -0.5

F32 = mybir.dt.float32
BF16 = mybir.dt.bfloat16
AF = mybir.ActivationFunctionType
ALU = mybir.AluOpType


def build_nc():
    nc = bacc.Bacc()

    xb = nc.dram_tensor("xb", [C, N], BF16, kind="ExternalInput")
    xq = nc.dram_tensor("xq", [C, NQ], F32, kind="ExternalInput")
    wq = nc.dram_tensor("wq", [C, C], BF16, kind="ExternalInput")  # [cin, cout]
    wk = nc.dram_tensor("wk", [C, C], BF16, kind="ExternalInput")  # RAW Wk
    wv = nc.dram_tensor("wv", [C, C], BF16, kind="ExternalInput")
    wo = nc.dram_tensor("wo", [C, C], BF16, kind="ExternalInput")
    gcol = nc.dram_tensor("gcol", [P, KB], F32, kind="ExternalInput")   # gamma
    bcol = nc.dram_tensor("bcol", [P, KB], F32, kind="ExternalInput")   # beta
    bqc = nc.dram_tensor("bqc", [P, KB], F32, kind="ExternalInput")     # bq*SCALE
    boc = nc.dram_tensor("boc", [P, KB], F32, kind="ExternalInput")     # bo
    bv = nc.dram_tensor("bv", [C], F32, kind="ExternalInput")
    gavg = nc.dram_tensor("gavg", [P, P], F32, kind="ExternalInput")
    out = nc.dram_tensor("out", [C, NQ], F32, kind="ExternalOutput")

    xb_r = xb[:].rearrange("(blk p) n -> p blk n", p=P)
    xq_r = xq[:].rearrange("(blk p) n -> p blk n", p=P)
    out_r = out[:].rearrange("(blk p) n -> p blk n", p=P)
    wq_r = wq[:].rearrange("(kb p) co -> p kb co", p=P)
    wk_r = wk[:].rearrange("(kb p) co -> p kb co", p=P)
    wv_r = wv[:].rearrange("(kb p) co -> p kb co", p=P)
    wo_r = wo[:].rearrange("(kb p) co -> p kb co", p=P)

    with tile.TileContext(nc) as tc:
        with (
            tc.tile_pool(name="big", bufs=1) as big,
            tc.tile_pool(name="st", bufs=1) as st,
            tc.tile_pool(name="et", bufs=8) as etp,
            tc.tile_pool(name="ep", bufs=2) as ep,
            tc.tile_pool(name="epo", bufs=4) as epo,
            tc.tile_pool(name="mm", bufs=3, space="PSUM") as psmm,
            tc.tile_pool(name="pvp", bufs=1, space="PSUM") as pvp,
        ):
            # ---- persistent tiles ----
            x_sb = big.tile([P, KB, N], BF16)    # x, normalized in place -> hn
            hq = big.tile([P, KB, NQ], BF16)
            vT = big.tile([P, JT, C], BF16)
            q_sb = big.tile([P, KB, NQ], BF16)
            q2_sb = big.tile([P, KB, NQ], BF16)
            xq_sb = big.tile([P, KB, NQ], F32)
            wq_sb = big.tile([P, KB, C], BF16)
            wk_sb = big.tile([P, KB, C], BF16)
            wv_sb = big.tile([P, KB, C], BF16)
            wo_sb = big.tile([P, KB, C], BF16)
            gcol_sb = big.tile([P, KB], F32)
            bcol_sb = big.tile([P, KB], F32)
            bqc_sb = big.tile([P, KB], F32)
            boc_sb = big.tile([P, KB], F32)
            bv_sb = big.tile([P, 512], F32)
            gavg_sb = big.tile([P, P], F32)
            ones_f = big.tile([P, 1], F32)
            e0 = big.tile([P, P], F32)      # row 0 = 1, else 0 (for row bcast)
            rpad = big.tile([P, 512], F32)  # row 0 = 1/s, else 0

            # x streams in first so bn_stats can start ASAP
            XPIECES = [(nt * 512, 512) for nt in range(NT)]
            for off, w in XPIECES:
                nc.sync.dma_start(out=x_sb[:, :, off:off + w],
                                  in_=xb_r[:, :, off:off + w])
            nc.sync.dma_start(out=wq_sb, in_=wq_r)
            nc.sync.dma_start(out=wk_sb, in_=wk_r)
            nc.sync.dma_start(out=wv_sb, in_=wv_r)
            nc.sync.dma_start(out=wo_sb, in_=wo_r)
            nc.sync.dma_start(out=gcol_sb, in_=gcol[:])
            nc.sync.dma_start(out=bcol_sb, in_=bcol[:])
            nc.sync.dma_start(out=bqc_sb, in_=bqc[:])
            nc.sync.dma_start(out=boc_sb, in_=boc[:])
            nc.sync.dma_start(out=gavg_sb, in_=gavg[:])
            # broadcast bv across all partitions (partition step 0)
            bv_ap = bv[:]
            nc.gpsimd.dma_start(
                out=bv_sb,
                in_=bass.AP(tensor=bv_ap.tensor, offset=bv_ap.offset,
                            ap=[[0, P], *bv_ap.ap]),
            )
            nc.vector.memset(ones_f, 1.0)
            nc.vector.memset(e0, 0.0)
            nc.vector.memset(e0[0:1, :], 1.0)
            nc.vector.memset(rpad, 0.0)
            nc.sync.dma_start(out=xq_sb, in_=xq_r)

            # ---- stage A: groupnorm stats ----
            # Split across engines: channel-block 0 of each chunk goes to ACT
            # (Copy/Square passes with accum_out -> per-partition sums),
            # blocks 1..3 go to DVE bn_stats. Both finish ~5us sooner than
            # DVE-alone. ACT passes are grouped by func to avoid table loads.
            NPC = len(XPIECES)
            stats = st.tile([P, KB - 1, NPC, 6], F32)
            adump = st.tile([P, 512], F32)
            accs = st.tile([P, NPC], F32)
            accq = st.tile([P, NPC], F32)
            for i, (off, w) in enumerate(XPIECES):
                nc.scalar.activation(out=adump[:, :w], in_=x_sb[:, 0, off:off + w],
                                     func=AF.Copy, accum_out=accs[:, i:i + 1])
                for kb in range(1, KB):
                    nc.vector.bn_stats(out=stats[:, kb - 1, i, :],
                                       in_=x_sb[:, kb, off:off + w])
            for i, (off, w) in enumerate(XPIECES):
                nc.scalar.activation(out=adump[:, :w], in_=x_sb[:, 0, off:off + w],
                                     func=AF.Square, accum_out=accq[:, i:i + 1])
            mv = st.tile([P, KB - 1, 2], F32)
            for kb in range(1, KB):
                nc.vector.bn_aggr(out=mv[:, kb - 1, :], in_=stats[:, kb - 1, :, :])

            # per-partition stats -> per-group stats -> per-channel A/D
            stat8 = st.tile([P, 8], F32)
            sm0 = st.tile([P, 2], F32)
            nc.vector.reduce_sum(out=sm0[:, 0:1], in_=accs,
                                 axis=mybir.AxisListType.X)
            nc.vector.reduce_sum(out=sm0[:, 1:2], in_=accq,
                                 axis=mybir.AxisListType.X)
            nc.vector.tensor_scalar_mul(sm0, sm0, 1.0 / float(N))
            nc.vector.tensor_copy(out=stat8[:, 0:1], in_=sm0[:, 0:1])
            nc.vector.tensor_copy(out=stat8[:, 4:5], in_=sm0[:, 1:2])
            nc.vector.tensor_copy(out=stat8[:, 1:4], in_=mv[:, :, 0])
            nc.vector.tensor_tensor(out=stat8[:, 5:8], in0=mv[:, :, 0],
                                    in1=mv[:, :, 0], op=ALU.mult)
            nc.vector.tensor_tensor(out=stat8[:, 5:8], in0=stat8[:, 5:8],
                                    in1=mv[:, :, 1], op=ALU.add)
            psb = psmm.tile([P, 8], F32, tag="mm", name="psb")
            nc.tensor.matmul(psb, gavg_sb, stat8, start=True, stop=True)
            mq = st.tile([P, 8], F32)
            nc.vector.tensor_copy(out=mq, in_=psb)
            varg = st.tile([P, 4], F32)
            nc.vector.tensor_tensor(out=varg, in0=mq[:, 0:4], in1=mq[:, 0:4],
                                    op=ALU.mult)
            nc.vector.tensor_tensor(out=varg, in0=mq[:, 4:8], in1=varg,
                                    op=ALU.subtract)
            rstd = st.tile([P, 4], F32)
            eps_sb = st.tile([P, 1], F32)
            nc.vector.memset(eps_sb, EPS)
            nc.scalar.activation(out=rstd, in_=varg, func=AF.Sqrt, bias=eps_sb)
            nc.vector.reciprocal(out=rstd, in_=rstd)
            expdump = st.tile([P, 1], F32)
            nc.scalar.activation(out=expdump, in_=eps_sb, func=AF.Exp)
            # A = rstd*gamma ; D = beta - mean*A  (hn = A*x + D)
            A = st.tile([P, 4], F32)
            D = st.tile([P, 4], F32)
            nc.vector.tensor_tensor(out=A, in0=rstd, in1=gcol_sb, op=ALU.mult)
            nc.vector.tensor_tensor(out=D, in0=mq[:, 0:4], in1=A, op=ALU.mult)
            nc.vector.tensor_tensor(out=D, in0=bcol_sb, in1=D, op=ALU.subtract)
            # ---- stage A/B interleaved: normalize x in place chunk-wise and
            # run k/vT projections on each chunk as soon as it is ready.
            for nt in range(NT):
                for kb in range(KB):
                    sl = x_sb[:, kb, nt * 512:(nt + 1) * 512]
                    nc.vector.tensor_scalar(
                        out=sl, in0=sl,
                        scalar1=A[:, kb:kb + 1], scalar2=D[:, kb:kb + 1],
                        op0=ALU.mult, op1=ALU.add)
                # vT for this chunk's 4 j-tiles
                for j4 in range(4):
                    jt = nt * 4 + j4
                    pv = psmm.tile([P, 512], F32, tag="mm", name="pv")
                    for kb in range(KB):
                        nc.tensor.matmul(
                            pv, x_sb[:, kb, jt * P:(jt + 1) * P], wv_sb[:, kb, :],
                            start=(kb == 0), stop=(kb == KB - 1))
                    nc.vector.tensor_tensor(out=vT[:, jt, :], in0=pv, in1=bv_sb,
                                            op=ALU.add)

            # hq: off the stats -> first-projection critical path
            Aq = st.tile([P, 4], F32)
            Dq = st.tile([P, 4], F32)
            nc.vector.tensor_scalar_mul(Aq, A, SCALE)
            nc.vector.tensor_scalar_mul(Dq, D, SCALE)
            for kb in range(KB):
                nc.vector.tensor_scalar(
                    out=hq[:, kb, :], in0=xq_sb[:, kb, :],
                    scalar1=Aq[:, kb:kb + 1], scalar2=Dq[:, kb:kb + 1],
                    op0=ALU.mult, op1=ALU.add)
            # fold the out-proj bias into the residual: xq_sb += bo (per chan)
            for kb in range(KB):
                nc.vector.tensor_scalar_add(
                    xq_sb[:, kb, :], xq_sb[:, kb, :], boc_sb[:, kb:kb + 1])
            # q = (Wq @ hq_scaled) + bq*SCALE  (SCALE folded into hq/bqc),
            # then q2 = Wk^T @ q: folds the k projection through the score
            # matmul (S^T = (Wk hn)^T q = hn^T (Wk^T q)); bk's contribution
            # is constant along the softmax axis and cancels exactly.
            # i2-half-major order so q2 of half 0 overlaps q of half 1.
            for i2 in range(IH):
                for blk in range(KB):
                    pq = psmm.tile([P, 512], F32, tag="mm", name="pq")
                    for kb in range(KB):
                        nc.tensor.matmul(
                            pq, wq_sb[:, kb, blk * P:(blk + 1) * P],
                            hq[:, kb, i2 * 512:(i2 + 1) * 512],
                            start=(kb == 0), stop=(kb == KB - 1))
                    nc.vector.tensor_scalar_add(
                        q_sb[:, blk, i2 * 512:(i2 + 1) * 512], pq,
                        bqc_sb[:, blk:blk + 1])
                for blk in range(KB):
                    p2 = psmm.tile([P, 512], F32, tag="mm", name="p2")
                    for kb in range(KB):
                        nc.tensor.matmul(
                            p2, wk_sb[:, kb, blk * P:(blk + 1) * P],
                            q_sb[:, kb, i2 * 512:(i2 + 1) * 512],
                            start=(kb == 0), stop=(kb == KB - 1))
                    nc.vector.tensor_copy(
                        out=q2_sb[:, blk, i2 * 512:(i2 + 1) * 512], in_=p2)

            # ---- stage C: attention, pipelined over (ih, jt) ----
            # Per step: scores+exp for (ih, jt); PV matmuls for the previous
            # step; DVE row-sum accumulation into sden_sb. The ih=0 epilogue
            # is emitted a few steps into ih=1 so PE never waits on the
            # reciprocal/broadcast chain.
            pv_ps = {}
            sden_sb = {}
            ets = {}

            def epilogue(ih):
                # Copy unnormalized PV to SBUF right away (frees the pv psum
                # accumulators for the next half) and run the Wo projection on
                # it; the softmax 1/s is folded in afterwards:
                #   out = (Wo @ PV)/s + bo + xq
                on = ep.tile([P, KB, 512], BF16, tag="on", name=f"on{ih}")
                for cc in range(KB):
                    nc.vector.tensor_copy(out=on[:, cc, :], in_=pv_ps[ih][cc])
                # cross-partition sum of sden_sb -> s[1, 512]; r = 1/s
                sden = pvp.tile([1, 512], F32, tag="sden", name=f"sden{ih}")
                nc.tensor.matmul(sden, ones_f, sden_sb[ih], start=True, stop=True)
                nc.vector.reciprocal(out=rpad[0:1, :], in_=sden)
                rb = psmm.tile([P, 512], F32, tag="mm", name=f"rb{ih}")
                nc.tensor.matmul(rb, e0, rpad, start=True, stop=True)
                rbs = ep.tile([P, 512], F32, tag="rbs", name=f"rbs{ih}")
                nc.vector.tensor_copy(out=rbs, in_=rb)
                for blk in range(KB):
                    po = psmm.tile([P, 512], F32, tag="mm", name=f"po{ih}")
                    for cc in range(KB):
                        nc.tensor.matmul(
                            po, wo_sb[:, cc, blk * P:(blk + 1) * P], on[:, cc, :],
                            start=(cc == 0), stop=(cc == KB - 1))
                    ot = epo.tile([P, 512], F32, tag="ot", name=f"ot{ih}")
                    nc.vector.tensor_tensor(out=ot, in0=po, in1=rbs, op=ALU.mult)
                    nc.vector.tensor_tensor(
                        out=ot, in0=ot,
                        in1=xq_sb[:, blk, ih * 512:(ih + 1) * 512], op=ALU.add)
                    nc.sync.dma_start(
                        out=out_r[:, blk, ih * 512:(ih + 1) * 512], in_=ot)

            NSTEP = IH * JT
            for step in range(NSTEP + 1):
                if step < NSTEP:
                    ih, jt = divmod(step, JT)
                    if jt == 0:
                        pv_ps[ih] = [pvp.tile([P, 512], F32, tag=f"pv{cc}",
                                              name=f"pv{ih}_{cc}")
                                     for cc in range(KB)]
                        sden_sb[ih] = ep.tile([P, 512], F32, tag="sd",
                                              name=f"sd{ih}")
                    ss = psmm.tile([P, 512], F32, tag="mm", name="ss")
                    for kb in range(KB):
                        nc.tensor.matmul(
                            ss, x_sb[:, kb, jt * P:(jt + 1) * P],
                            q2_sb[:, kb, ih * 512:(ih + 1) * 512],
                            start=(kb == 0), stop=(kb == KB - 1))
                    et = etp.tile([P, 512], BF16, tag="et", name="et")
                    nc.scalar.activation(out=et, in_=ss, func=AF.Exp)
                    ets[step] = et
                    if jt == 0:
                        nc.vector.tensor_copy(out=sden_sb[ih], in_=et)
                    else:
                        nc.vector.tensor_tensor(out=sden_sb[ih], in0=sden_sb[ih],
                                                in1=et, op=ALU.add)
                if step >= 1:
                    pih, pjt = divmod(step - 1, JT)
                    et = ets.pop(step - 1)
                    for cc in range(KB):
                        nc.tensor.matmul(
                            pv_ps[pih][cc], vT[:, pjt, cc * P:(cc + 1) * P],
                            et, start=(pjt == 0), stop=(pjt == JT - 1))
                if step == JT + 2:
                    epilogue(0)
            epilogue(1)

    nc.finalize()
    return nc


_NC = None


def _get_nc():
    global _NC
    if _NC is None:
        _NC = build_nc()
    return _NC


def _col(v):
    """[C] f32 -> [P, KB] with c = blk*128 + p."""
    return np.ascontiguousarray(np.asarray(v, np.float32).reshape(KB, P).T)


def _make_in_maps(inputs):
    x = np.asarray(inputs["x"], np.float32).reshape(2, C, N)
    x_bf = x.astype(ml_dtypes.bfloat16)
    wqT = np.ascontiguousarray(np.asarray(inputs["Wq"], np.float32).T).astype(ml_dtypes.bfloat16)
    wkR = np.ascontiguousarray(np.asarray(inputs["Wk"], np.float32)).astype(ml_dtypes.bfloat16)
    wvT = np.ascontiguousarray(np.asarray(inputs["Wv"], np.float32).T).astype(ml_dtypes.bfloat16)
    woT = np.ascontiguousarray(np.asarray(inputs["Wo"], np.float32).T).astype(ml_dtypes.bfloat16)
    gcol = _col(inputs["gamma"])
    bcol = _col(inputs["beta"])
    bqc = _col(np.asarray(inputs["bq"], np.float32) * SCALE)
    boc = _col(inputs["bo"])
    bvv = np.ascontiguousarray(np.asarray(inputs["bv"], np.float32))

    pidx = np.arange(P)
    gavg = np.where(pidx[:, None] // 16 == pidx[None, :] // 16,
                    np.float32(1.0 / 16.0), np.float32(0.0))

    common = dict(wq=wqT, wk=wkR, wv=wvT, wo=woT, gcol=gcol, bcol=bcol,
                  bqc=bqc, boc=boc, bv=bvv, gavg=gavg)
    in_maps = []
    for core in range(8):
        b, qc = core // 4, core % 4
        in_maps.append(dict(
            common,
            xb=np.ascontiguousarray(x_bf[b]),
            xq=np.ascontiguousarray(x[b][:, qc * NQ:(qc + 1) * NQ]),
        ))
    return in_maps


def run(inputs, trace=False):
    nc = _get_nc()
    in_maps = _make_in_maps(inputs)
    res = run_bass_kernel_spmd(nc, in_maps, core_ids=list(range(8)), trace=trace)
    y = np.empty((2, C, N), np.float32)
    for core in range(8):
        b, qc = core // 4, core % 4
        y[b][:, qc * NQ:(qc + 1) * NQ] = res.results[core]["out"]
    return y.reshape(2, C, 64, 64), res


def kernel(**inputs):
    y, _ = run(inputs, trace=False)
    return y


# revision 29
# speedup vs baseline: 1.0973x; 1.0018x over previous
"""AttnBlock (GroupNorm + single-head attention over HW + residual) on 8 trn2 cores.

Sharding: core = b*4 + qc  (b in 0..1 batch, qc in 0..3 query-column chunk).
Each core receives the full batch element x[b] ([512, 4096], pre-cast bf16)
plus its query chunk x[b][:, qc*1024:(qc+1)*1024] (f32), computes groupnorm +
k/v over all tokens (redundantly per batch) and attention/out-proj for its
1024 query rows.

Layout conventions (everything keyed off channel c = blk*128 + p):
  x/hn, k, q SBUF tiles: [p=128, blk=4, tokens]   (c on partitions)
  vT: [p=128 (token within j-tile), jt=32, c=512] (tokens on partitions)
Attention is computed transposed (S^T[j, i]) so that no on-chip transpose is
ever required: S^T = k(c,j-tile)^T x q(c,i); softmax row-sums accumulate on
DVE over j-tiles and are reduced across partitions with one ones-matmul; the
1/s row is broadcast to 128 partitions with one padded matmul.
"""

import numpy as np
import ml_dtypes

import concourse.bass as bass
import concourse.bacc as bacc
import concourse.mybir as mybir
import concourse.tile as tile
from concourse.bass_utils import run_bass_kernel_spmd

P = 128
C = 512
N = 4096          # tokens per batch element (H*W)
NQ = 1024         # query tokens per core
KB = C // P       # 4 channel blocks
NT = N // 512     # 8 token tiles of 512
JT = N // P       # 32 j tiles of 128
IH = NQ // 512    # 2 query halves of 512
EPS = 1e-6
SCALE = float(C) ** -0.5

F32 = mybir.dt.float32
BF16 = mybir.dt.bfloat16
AF = mybir.ActivationFunctionType
ALU = mybir.AluOpType


def build_nc():
    nc = bacc.Bacc()

    xb = nc.dram_tensor("xb", [C, N], BF16, kind="ExternalInput")
    xq = nc.dram_tensor("xq", [C, NQ], F32, kind="ExternalInput")
    wq = nc.dram_tensor("wq", [C, C], BF16, kind="ExternalInput")  # [cin, cout]
    wk = nc.dram_tensor("wk", [C, C], BF16, kind="ExternalInput")  # RAW Wk
    wv = nc.dram_tensor("wv", [C, C], BF16, kind="ExternalInput")
    wo = nc.dram_tensor("wo", [C, C], BF16, kind="ExternalInput")
    gcol = nc.dram_tensor("gcol", [P, KB], F32, kind="ExternalInput")   # gamma
    bcol = nc.dram_tensor("bcol", [P, KB], F32, kind="ExternalInput")   # beta
    bqc = nc.dram_tensor("bqc", [P, KB], F32, kind="ExternalInput")     # bq*SCALE
    boc = nc.dram_tensor("boc", [P, KB], F32, kind="ExternalInput")     # bo
    bv = nc.dram_tensor("bv", [C], F32, kind="ExternalInput")
    gavg = nc.dram_tensor("gavg", [P, P], F32, kind="ExternalInput")
    out = nc.dram_tensor("out", [C, NQ], F32, kind="ExternalOutput")

    xb_r = xb[:].rearrange("(blk p) n -> p blk n", p=P)
    xq_r = xq[:].rearrange("(blk p) n -> p blk n", p=P)
    out_r = out[:].rearrange("(blk p) n -> p blk n", p=P)
    wq_r = wq[:].rearrange("(kb p) co -> p kb co", p=P)
    wk_r = wk[:].rearrange("(kb p) co -> p kb co", p=P)
    wv_r = wv[:].rearrange("(kb p) co -> p kb co", p=P)
    wo_r = wo[:].rearrange("(kb p) co -> p kb co", p=P)

    with tile.TileContext(nc) as tc:
        with (
            tc.tile_pool(name="big", bufs=1) as big,
            tc.tile_pool(name="st", bufs=1) as st,
            tc.tile_pool(name="et", bufs=8) as etp,
            tc.tile_pool(name="ep", bufs=2) as ep,
            tc.tile_pool(name="epo", bufs=4) as epo,
            tc.tile_pool(name="mm", bufs=3, space="PSUM") as psmm,
            tc.tile_pool(name="pvp", bufs=1, space="PSUM") as pvp,
        ):
            # ---- persistent tiles ----
            x_sb = big.tile([P, KB, N], BF16)    # x, normalized in place -> hn
            hq = big.tile([P, KB, NQ], BF16)
            vT = big.tile([P, JT, C], BF16)
            q_sb = big.tile([P, KB, NQ], BF16)
            q2_sb = big.tile([P, KB, NQ], BF16)
            xq_sb = big.tile([P, KB, NQ], F32)
            wq_sb = big.tile([P, KB, C], BF16)
            wk_sb = big.tile([P, KB, C], BF16)
            wv_sb = big.tile([P, KB, C], BF16)
            wo_sb = big.tile([P, KB, C], BF16)
            gcol_sb = big.tile([P, KB], F32)
            bcol_sb = big.tile([P, KB], F32)
            bqc_sb = big.tile([P, KB], F32)
            boc_sb = big.tile([P, KB], F32)
            bv_sb = big.tile([P, 512], F32)
            gavg_sb = big.tile([P, P], F32)
            ones_f = big.tile([P, 1], F32)
            e0 = big.tile([P, P], F32)      # row 0 = 1, else 0 (for row bcast)
            rpad = big.tile([P, 512], F32)  # row 0 = 1/s, else 0

            # x streams in first so bn_stats can start ASAP
            XPIECES = [(nt * 512, 512) for nt in range(NT)]
            for off, w in XPIECES:
                nc.sync.dma_start(out=x_sb[:, :, off:off + w],
                                  in_=xb_r[:, :, off:off + w])
            nc.sync.dma_start(out=wq_sb, in_=wq_r)
            nc.sync.dma_start(out=wk_sb, in_=wk_r)
            nc.sync.dma_start(out=wv_sb, in_=wv_r)
            nc.sync.dma_start(out=wo_sb, in_=wo_r)
            nc.sync.dma_start(out=gcol_sb, in_=gcol[:])
            nc.sync.dma_start(out=bcol_sb, in_=bcol[:])
            nc.sync.dma_start(out=bqc_sb, in_=bqc[:])
            nc.sync.dma_start(out=boc_sb, in_=boc[:])
            nc.sync.dma_start(out=gavg_sb, in_=gavg[:])
            # broadcast bv across all partitions (partition step 0)
            bv_ap = bv[:]
            nc.gpsimd.dma_start(
                out=bv_sb,
                in_=bass.AP(tensor=bv_ap.tensor, offset=bv_ap.offset,
                            ap=[[0, P], *bv_ap.ap]),
            )
            nc.vector.memset(ones_f, 1.0)
            nc.vector.memset(e0, 0.0)
            nc.vector.memset(e0[0:1, :], 1.0)
            nc.vector.memset(rpad, 0.0)
            nc.sync.dma_start(out=xq_sb, in_=xq_r)

            # ---- stage A: groupnorm stats ----
            # Split across engines: channel-block 0 of each chunk goes to ACT
            # (Copy/Square passes with accum_out -> per-partition sums),
            # blocks 1..3 go to DVE bn_stats. Both finish ~5us sooner than
            # DVE-alone. ACT passes are grouped by func to avoid table loads.
            NPC = len(XPIECES)
            stats = st.tile([P, KB - 1, NPC, 6], F32)
            adump = st.tile([P, 512], F32)
            accs = st.tile([P, NPC], F32)
            accq = st.tile([P, NPC], F32)
            for i, (off, w) in enumerate(XPIECES):
                nc.scalar.activation(out=adump[:, :w], in_=x_sb[:, 0, off:off + w],
                                     func=AF.Copy, accum_out=accs[:, i:i + 1])
                for kb in range(1, KB):
                    nc.vector.bn_stats(out=stats[:, kb - 1, i, :],
                                       in_=x_sb[:, kb, off:off + w])
            for i, (off, w) in enumerate(XPIECES):
                nc.scalar.activation(out=adump[:, :w], in_=x_sb[:, 0, off:off + w],
                                     func=AF.Square, accum_out=accq[:, i:i + 1])
            mv = st.tile([P, KB - 1, 2], F32)
            for kb in range(1, KB):
                nc.vector.bn_aggr(out=mv[:, kb - 1, :], in_=stats[:, kb - 1, :, :])

            # per-partition stats -> per-group stats -> per-channel A/D
            stat8 = st.tile([P, 8], F32)
            sm0 = st.tile([P, 2], F32)
            nc.vector.reduce_sum(out=sm0[:, 0:1], in_=accs,
                                 axis=mybir.AxisListType.X)
            nc.vector.reduce_sum(out=sm0[:, 1:2], in_=accq,
                                 axis=mybir.AxisListType.X)
            nc.vector.tensor_scalar_mul(sm0, sm0, 1.0 / float(N))
            nc.vector.tensor_copy(out=stat8[:, 0:1], in_=sm0[:, 0:1])
            nc.vector.tensor_copy(out=stat8[:, 4:5], in_=sm0[:, 1:2])
            nc.vector.tensor_copy(out=stat8[:, 1:4], in_=mv[:, :, 0])
            nc.vector.tensor_tensor(out=stat8[:, 5:8], in0=mv[:, :, 0],
                                    in1=mv[:, :, 0], op=ALU.mult)
            nc.vector.tensor_tensor(out=stat8[:, 5:8], in0=stat8[:, 5:8],
                                    in1=mv[:, :, 1], op=ALU.add)
            psb = psmm.tile([P, 8], F32, tag="mm", name="psb")
            nc.tensor.matmul(psb, gavg_sb, stat8, start=True, stop=True)
            mq = st.tile([P, 8], F32)
            nc.vector.tensor_copy(out=mq, in_=psb)
            varg = st.tile([P, 4], F32)
            nc.vector.tensor_tensor(out=varg, in0=mq[:, 0:4], in1=mq[:, 0:4],
                                    op=ALU.mult)
            nc.vector.tensor_tensor(out=varg, in0=mq[:, 4:8], in1=varg,
                                    op=ALU.subtract)
            rstd = st.tile([P, 4], F32)
            eps_sb = st.tile([P, 1], F32)
            nc.vector.memset(eps_sb, EPS)
            nc.scalar.activation(out=rstd, in_=varg, func=AF.Sqrt, bias=eps_sb)
            nc.vector.reciprocal(out=rstd, in_=rstd)
            expdump = st.tile([P, 1], F32)
            nc.scalar.activation(out=expdump, in_=eps_sb, func=AF.Exp)
            # A = rstd*gamma ; D = beta - mean*A  (hn = A*x + D)
            A = st.tile([P, 4], F32)
            D = st.tile([P, 4], F32)
            nc.vector.tensor_tensor(out=A, in0=rstd, in1=gcol_sb, op=ALU.mult)
            nc.vector.tensor_tensor(out=D, in0=mq[:, 0:4], in1=A, op=ALU.mult)
            nc.vector.tensor_tensor(out=D, in0=bcol_sb, in1=D, op=ALU.subtract)
            # x is NEVER normalized: hn = A*x + D is folded instead —
            #   scores: S^T = x^T (A*q2) + const(i); const cancels in softmax
            #   vT:     Wv' = A*WvT (rows), bias row += D @ WvT
            D_bf = st.tile([P, 4], BF16)
            nc.vector.tensor_copy(out=D_bf, in_=D)
            pbv = psmm.tile([1, 512], F32, tag="mm", name="pbv")
            for kb in range(KB):
                nc.tensor.matmul(pbv, D_bf[:, kb:kb + 1], wv_sb[:, kb, :],
                                 start=(kb == 0), stop=(kb == KB - 1))
            nc.vector.tensor_copy(out=rpad[0:1, :], in_=pbv)
            pbvb = psmm.tile([P, 512], F32, tag="mm", name="pbvb")
            nc.tensor.matmul(pbvb, e0, rpad, start=True, stop=True)
            nc.vector.tensor_tensor(out=bv_sb, in0=bv_sb, in1=pbvb, op=ALU.add)
            for kb in range(KB):
                nc.vector.tensor_scalar_mul(wv_sb[:, kb, :], wv_sb[:, kb, :],
                                            A[:, kb:kb + 1])
            # ---- stage B: vT projection straight off RAW x (A/D folded)
            for nt in range(NT):
                # vT for this chunk's 4 j-tiles
                for j4 in range(4):
                    jt = nt * 4 + j4
                    pv = psmm.tile([P, 512], F32, tag="mm", name="pv")
                    for kb in range(KB):
                        nc.tensor.matmul(
                            pv, x_sb[:, kb, jt * P:(jt + 1) * P], wv_sb[:, kb, :],
                            start=(kb == 0), stop=(kb == KB - 1))
                    nc.vector.tensor_tensor(out=vT[:, jt, :], in0=pv, in1=bv_sb,
                                            op=ALU.add)

            # hq: off the stats -> first-projection critical path
            Aq = st.tile([P, 4], F32)
            Dq = st.tile([P, 4], F32)
            nc.vector.tensor_scalar_mul(Aq, A, SCALE)
            nc.vector.tensor_scalar_mul(Dq, D, SCALE)
            for kb in range(KB):
                nc.vector.tensor_scalar(
                    out=hq[:, kb, :], in0=xq_sb[:, kb, :],
                    scalar1=Aq[:, kb:kb + 1], scalar2=Dq[:, kb:kb + 1],
                    op0=ALU.mult, op1=ALU.add)
            # fold the out-proj bias into the residual: xq_sb += bo (per chan)
            for kb in range(KB):
                nc.vector.tensor_scalar_add(
                    xq_sb[:, kb, :], xq_sb[:, kb, :], boc_sb[:, kb:kb + 1])
            # q = (Wq @ hq_scaled) + bq*SCALE  (SCALE folded into hq/bqc),
            # then q2 = Wk^T @ q: folds the k projection through the score
            # matmul (S^T = (Wk hn)^T q = hn^T (Wk^T q)); bk's contribution
            # is constant along the softmax axis and cancels exactly.
            # i2-half-major order so q2 of half 0 overlaps q of half 1.
            for i2 in range(IH):
                for blk in range(KB):
                    pq = psmm.tile([P, 512], F32, tag="mm", name="pq")
                    for kb in range(KB):
                        nc.tensor.matmul(
                            pq, wq_sb[:, kb, blk * P:(blk + 1) * P],
                            hq[:, kb, i2 * 512:(i2 + 1) * 512],
                            start=(kb == 0), stop=(kb == KB - 1))
                    nc.vector.tensor_scalar_add(
                        q_sb[:, blk, i2 * 512:(i2 + 1) * 512], pq,
                        bqc_sb[:, blk:blk + 1])
                for blk in range(KB):
                    p2 = psmm.tile([P, 512], F32, tag="mm", name="p2")
                    for kb in range(KB):
                        nc.tensor.matmul(
                            p2, wk_sb[:, kb, blk * P:(blk + 1) * P],
                            q_sb[:, kb, i2 * 512:(i2 + 1) * 512],
                            start=(kb == 0), stop=(kb == KB - 1))
                    nc.vector.tensor_scalar_mul(
                        q2_sb[:, blk, i2 * 512:(i2 + 1) * 512], p2,
                        A[:, blk:blk + 1])

            # ---- stage C: attention, pipelined over (ih, jt) ----
            # Per step: scores+exp for (ih, jt); PV matmuls for the previous
            # step; DVE row-sum accumulation into sden_sb. The ih=0 epilogue
            # is emitted a few steps into ih=1 so PE never waits on the
            # reciprocal/broadcast chain.
            pv_ps = {}
            sden_sb = {}
            ets = {}

            def epilogue(ih):
                # Copy unnormalized PV to SBUF right away (frees the pv psum
                # accumulators for the next half) and run the Wo projection on
                # it; the softmax 1/s is folded in afterwards:
                #   out = (Wo @ PV)/s + bo + xq
                on = ep.tile([P, KB, 512], BF16, tag="on", name=f"on{ih}")
                for cc in range(KB):
                    nc.vector.tensor_copy(out=on[:, cc, :], in_=pv_ps[ih][cc])
                # cross-partition sum of sden_sb -> s[1, 512]; r = 1/s
                sden = pvp.tile([1, 512], F32, tag="sden", name=f"sden{ih}")
                nc.tensor.matmul(sden, ones_f, sden_sb[ih], start=True, stop=True)
                nc.vector.reciprocal(out=rpad[0:1, :], in_=sden)
                rb = psmm.tile([P, 512], F32, tag="mm", name=f"rb{ih}")
                nc.tensor.matmul(rb, e0, rpad, start=True, stop=True)
                rbs = ep.tile([P, 512], F32, tag="rbs", name=f"rbs{ih}")
                nc.vector.tensor_copy(out=rbs, in_=rb)
                for blk in range(KB):
                    po = psmm.tile([P, 512], F32, tag="mm", name=f"po{ih}")
                    for cc in range(KB):
                        nc.tensor.matmul(
                            po, wo_sb[:, cc, blk * P:(blk + 1) * P], on[:, cc, :],
                            start=(cc == 0), stop=(cc == KB - 1))
                    ot = epo.tile([P, 512], F32, tag="ot", name=f"ot{ih}")
                    nc.vector.tensor_tensor(out=ot, in0=po, in1=rbs, op=ALU.mult)
                    nc.vector.tensor_tensor(
                        out=ot, in0=ot,
                        in1=xq_sb[:, blk, ih * 512:(ih + 1) * 512], op=ALU.add)
                    nc.sync.dma_start(
                        out=out_r[:, blk, ih * 512:(ih + 1) * 512], in_=ot)

            NSTEP = IH * JT
            for step in range(NSTEP + 1):
                if step < NSTEP:
                    ih, jt = divmod(step, JT)
                    if jt == 0:
                        pv_ps[ih] = [pvp.tile([P, 512], F32, tag=f"pv{cc}",
                                              name=f"pv{ih}_{cc}")
                                     for cc in range(KB)]
                        sden_sb[ih] = ep.tile([P, 512], F32, tag="sd",
                                              name=f"sd{ih}")
                    ss = psmm.tile([P, 512], F32, tag="mm", name="ss")
                    for kb in range(KB):
                        nc.tensor.matmul(
                            ss, x_sb[:, kb, jt * P:(jt + 1) * P],
                            q2_sb[:, kb, ih * 512:(ih + 1) * 512],
                            start=(kb == 0), stop=(kb == KB - 1))
                    et = etp.tile([P, 512], BF16, tag="et", name="et")
                    nc.scalar.activation(out=et, in_=ss, func=AF.Exp)
                    ets[step] = et
                    if jt == 0:
                        nc.vector.tensor_copy(out=sden_sb[ih], in_=et)
                    else:
                        nc.vector.tensor_tensor(out=sden_sb[ih], in0=sden_sb[ih],
                                                in1=et, op=ALU.add)
                if step >= 1:
                    pih, pjt = divmod(step - 1, JT)
                    et = ets.pop(step - 1)
                    for cc in range(KB):
                        nc.tensor.matmul(
                            pv_ps[pih][cc], vT[:, pjt, cc * P:(cc + 1) * P],
                            et, start=(pjt == 0), stop=(pjt == JT - 1))
                if step == JT + 2:
                    epilogue(0)
            epilogue(1)

    nc.finalize()
    return nc


_NC = None


def _get_nc():
    global _NC
    if _NC is None:
        _NC = build_nc()
    return _NC


def _col(v):
    """[C] f32 -> [P, KB] with c = blk*128 + p."""
    return np.ascontiguousarray(np.asarray(v, np.float32).reshape(KB, P).T)


def _make_in_maps(inputs):
    x = np.asarray(inputs["x"], np.float32).reshape(2, C, N)
    x_bf = x.astype(ml_dtypes.bfloat16)
    wqT = np.ascontiguousarray(np.asarray(inputs["Wq"], np.float32).T).astype(ml_dtypes.bfloat16)
    wkR = np.ascontiguousarray(np.asarray(inputs["Wk"], np.float32)).astype(ml_dtypes.bfloat16)
    wvT = np.ascontiguousarray(np.asarray(inputs["Wv"], np.float32).T).astype(ml_dtypes.bfloat16)
    woT = np.ascontiguousarray(np.asarray(inputs["Wo"], np.float32).T).astype(ml_dtypes.bfloat16)
    gcol = _col(inputs["gamma"])
    bcol = _col(inputs["beta"])
    bqc = _col(np.asarray(inputs["bq"], np.float32) * SCALE)
    boc = _col(inputs["bo"])
    bvv = np.ascontiguousarray(np.asarray(inputs["bv"], np.float32))

    pidx = np.arange(P)
    gavg = np.where(pidx[:, None] // 16 == pidx[None, :] // 16,
                    np.float32(1.0 / 16.0), np.float32(0.0))

    common = dict(wq=wqT, wk=wkR, wv=wvT, wo=woT, gcol=gcol, bcol=bcol,
                  bqc=bqc, boc=boc, bv=bvv, gavg=gavg)
    in_maps = []
    for core in range(8):
        b, qc = core // 4, core % 4
        in_maps.append(dict(
            common,
            xb=np.ascontiguousarray(x_bf[b]),
            xq=np.ascontiguousarray(x[b][:, qc * NQ:(qc + 1) * NQ]),
        ))
    return in_maps


def run(inputs, trace=False):
    nc = _get_nc()
    in_maps = _make_in_maps(inputs)
    res = run_bass_kernel_spmd(nc, in_maps, core_ids=list(range(8)), trace=trace)
    y = np.empty((2, C, N), np.float32)
    for core in range(8):
        b, qc = core // 4, core % 4
        y[b][:, qc * NQ:(qc + 1) * NQ] = res.results[core]["out"]
    return y.reshape(2, C, 64, 64), res


def kernel(**inputs):
    y, _ = run(inputs, trace=False)
    return y


# revision 32
# speedup vs baseline: 1.0974x; 1.0001x over previous
"""AttnBlock (GroupNorm + single-head attention over HW + residual) on 8 trn2 cores.

Sharding: core = b*4 + qc  (b in 0..1 batch, qc in 0..3 query-column chunk).
Each core receives the full batch element x[b] ([512, 4096], pre-cast bf16)
plus its query chunk x[b][:, qc*1024:(qc+1)*1024] (f32), computes groupnorm +
k/v over all tokens (redundantly per batch) and attention/out-proj for its
1024 query rows.

Layout conventions (everything keyed off channel c = blk*128 + p):
  x/hn, k, q SBUF tiles: [p=128, blk=4, tokens]   (c on partitions)
  vT: [p=128 (token within j-tile), jt=32, c=512] (tokens on partitions)
Attention is computed transposed (S^T[j, i]) so that no on-chip transpose is
ever required: S^T = k(c,j-tile)^T x q(c,i); softmax row-sums accumulate on
DVE over j-tiles and are reduced across partitions with one ones-matmul; the
1/s row is broadcast to 128 partitions with one padded matmul.
"""

import numpy as np
import ml_dtypes

import concourse.bass as bass
import concourse.bacc as bacc
import concourse.mybir as mybir
import concourse.tile as tile
from concourse.bass_utils import run_bass_kernel_spmd

P = 128
C = 512
N = 4096          # tokens per batch element (H*W)
NQ = 1024         # query tokens per core
KB = C // P       # 4 channel blocks
NT = N // 512     # 8 token tiles of 512
JT = N // P       # 32 j tiles of 128
IH = NQ // 512    # 2 query halves of 512
EPS = 1e-6
SCALE = float(C) ** -0.5

F32 = mybir.dt.float32
BF16 = mybir.dt.bfloat16
AF = mybir.ActivationFunctionType
ALU = mybir.AluOpType


def build_nc():
    nc = bacc.Bacc()

    xb = nc.dram_tensor("xb", [C, N], BF16, kind="ExternalInput")
    xq = nc.dram_tensor("xq", [C, NQ], F32, kind="ExternalInput")
    wq = nc.dram_tensor("wq", [C, C], BF16, kind="ExternalInput")  # [cin, cout]
    wk = nc.dram_tensor("wk", [C, C], BF16, kind="ExternalInput")  # RAW Wk
    wv = nc.dram_tensor("wv", [C, C], BF16, kind="ExternalInput")
    wo = nc.dram_tensor("wo", [C, C], BF16, kind="ExternalInput")
    gcol = nc.dram_tensor("gcol", [P, KB], F32, kind="ExternalInput")   # gamma
    bcol = nc.dram_tensor("bcol", [P, KB], F32, kind="ExternalInput")   # beta
    bqc = nc.dram_tensor("bqc", [P, KB], F32, kind="ExternalInput")     # bq*SCALE
    boc = nc.dram_tensor("boc", [P, KB], F32, kind="ExternalInput")     # bo
    bv = nc.dram_tensor("bv", [C], F32, kind="ExternalInput")
    gavg = nc.dram_tensor("gavg", [P, P], F32, kind="ExternalInput")
    out = nc.dram_tensor("out", [C, NQ], F32, kind="ExternalOutput")

    xb_r = xb[:].rearrange("(blk p) n -> p blk n", p=P)
    xq_r = xq[:].rearrange("(blk p) n -> p blk n", p=P)
    out_r = out[:].rearrange("(blk p) n -> p blk n", p=P)
    wq_r = wq[:].rearrange("(kb p) co -> p kb co", p=P)
    wk_r = wk[:].rearrange("(kb p) co -> p kb co", p=P)
    wv_r = wv[:].rearrange("(kb p) co -> p kb co", p=P)
    wo_r = wo[:].rearrange("(kb p) co -> p kb co", p=P)

    with tile.TileContext(nc) as tc:
        with (
            tc.tile_pool(name="big", bufs=1) as big,
            tc.tile_pool(name="st", bufs=1) as st,
            tc.tile_pool(name="et", bufs=8) as etp,
            tc.tile_pool(name="ep", bufs=2) as ep,
            tc.tile_pool(name="epo", bufs=4) as epo,
            tc.tile_pool(name="mm", bufs=3, space="PSUM") as psmm,
            tc.tile_pool(name="pvp", bufs=1, space="PSUM") as pvp,
        ):
            # ---- persistent tiles ----
            x_sb = big.tile([P, KB, N], BF16)    # x, normalized in place -> hn
            hq = big.tile([P, KB, NQ], BF16)
            vT = big.tile([P, JT, C], BF16)
            q_sb = big.tile([P, KB, NQ], BF16)
            q2_sb = big.tile([P, KB, NQ], BF16)
            xq_sb = big.tile([P, KB, NQ], F32)
            wq_sb = big.tile([P, KB, C], BF16)
            wk_sb = big.tile([P, KB, C], BF16)
            wv_sb = big.tile([P, KB, C], BF16)
            wo_sb = big.tile([P, KB, C], BF16)
            gcol_sb = big.tile([P, KB], F32)
            bcol_sb = big.tile([P, KB], F32)
            bqc_sb = big.tile([P, KB], F32)
            boc_sb = big.tile([P, KB], F32)
            bv_sb = big.tile([P, 512], F32)
            gavg_sb = big.tile([P, P], F32)
            ones_f = big.tile([P, 1], F32)
            e0 = big.tile([P, P], F32)      # row 0 = 1, else 0 (for row bcast)
            rpad = big.tile([P, 512], F32)  # row 0 = 1/s, else 0

            # x streams in first so bn_stats can start ASAP
            XPIECES = [(nt * 512, 512) for nt in range(NT)]
            for off, w in XPIECES:
                nc.sync.dma_start(out=x_sb[:, :, off:off + w],
                                  in_=xb_r[:, :, off:off + w])
            nc.sync.dma_start(out=wq_sb, in_=wq_r)
            nc.sync.dma_start(out=wk_sb, in_=wk_r)
            nc.sync.dma_start(out=wv_sb, in_=wv_r)
            nc.sync.dma_start(out=wo_sb, in_=wo_r)
            nc.sync.dma_start(out=gcol_sb, in_=gcol[:])
            nc.sync.dma_start(out=bcol_sb, in_=bcol[:])
            nc.sync.dma_start(out=bqc_sb, in_=bqc[:])
            nc.sync.dma_start(out=boc_sb, in_=boc[:])
            nc.sync.dma_start(out=gavg_sb, in_=gavg[:])
            # broadcast bv across all partitions (partition step 0)
            bv_ap = bv[:]
            nc.gpsimd.dma_start(
                out=bv_sb,
                in_=bass.AP(tensor=bv_ap.tensor, offset=bv_ap.offset,
                            ap=[[0, P], *bv_ap.ap]),
            )
            nc.vector.memset(ones_f, 1.0)
            nc.vector.memset(e0, 0.0)
            nc.vector.memset(e0[0:1, :], 1.0)
            nc.vector.memset(rpad, 0.0)
            nc.sync.dma_start(out=xq_sb, in_=xq_r)

            # ---- stage A: groupnorm stats ----
            # Split across engines: channel-block 0 of each chunk goes to ACT
            # (Copy/Square passes with accum_out -> per-partition sums),
            # blocks 1..3 go to DVE bn_stats. Both finish ~5us sooner than
            # DVE-alone. ACT passes are grouped by func to avoid table loads.
            NPC = len(XPIECES)
            stats = st.tile([P, KB - 1, NPC, 6], F32)
            adump = st.tile([P, 512], F32)
            accs = st.tile([P, NPC], F32)
            accq = st.tile([P, NPC], F32)
            for i, (off, w) in enumerate(XPIECES):
                nc.scalar.activation(out=adump[:, :w], in_=x_sb[:, 0, off:off + w],
                                     func=AF.Copy, accum_out=accs[:, i:i + 1])
                nc.scalar.activation(out=adump[:, :w], in_=x_sb[:, 0, off:off + w],
                                     func=AF.Square, accum_out=accq[:, i:i + 1])
                for kb in range(1, KB):
                    nc.vector.bn_stats(out=stats[:, kb - 1, i, :],
                                       in_=x_sb[:, kb, off:off + w])
            mv = st.tile([P, KB - 1, 2], F32)
            for kb in range(1, KB):
                nc.vector.bn_aggr(out=mv[:, kb - 1, :], in_=stats[:, kb - 1, :, :])

            # per-partition stats -> per-group stats -> per-channel A/D
            stat8 = st.tile([P, 8], F32)
            sm0 = st.tile([P, 2], F32)
            nc.vector.reduce_sum(out=sm0[:, 0:1], in_=accs,
                                 axis=mybir.AxisListType.X)
            nc.vector.reduce_sum(out=sm0[:, 1:2], in_=accq,
                                 axis=mybir.AxisListType.X)
            nc.vector.tensor_scalar_mul(sm0, sm0, 1.0 / float(N))
            nc.vector.tensor_copy(out=stat8[:, 0:1], in_=sm0[:, 0:1])
            nc.vector.tensor_copy(out=stat8[:, 4:5], in_=sm0[:, 1:2])
            nc.vector.tensor_copy(out=stat8[:, 1:4], in_=mv[:, :, 0])
            nc.vector.tensor_tensor(out=stat8[:, 5:8], in0=mv[:, :, 0],
                                    in1=mv[:, :, 0], op=ALU.mult)
            nc.vector.tensor_tensor(out=stat8[:, 5:8], in0=stat8[:, 5:8],
                                    in1=mv[:, :, 1], op=ALU.add)
            psb = psmm.tile([P, 8], F32, tag="mm", name="psb")
            nc.tensor.matmul(psb, gavg_sb, stat8, start=True, stop=True)
            mq = st.tile([P, 8], F32)
            nc.vector.tensor_copy(out=mq, in_=psb)
            varg = st.tile([P, 4], F32)
            nc.vector.tensor_tensor(out=varg, in0=mq[:, 0:4], in1=mq[:, 0:4],
                                    op=ALU.mult)
            nc.vector.tensor_tensor(out=varg, in0=mq[:, 4:8], in1=varg,
                                    op=ALU.subtract)
            rstd = st.tile([P, 4], F32)
            eps_sb = st.tile([P, 1], F32)
            nc.vector.memset(eps_sb, EPS)
            nc.scalar.activation(out=rstd, in_=varg, func=AF.Sqrt, bias=eps_sb)
            nc.vector.reciprocal(out=rstd, in_=rstd)
            expdump = st.tile([P, 1], F32)
            nc.scalar.activation(out=expdump, in_=eps_sb, func=AF.Exp)
            # A = rstd*gamma ; D = beta - mean*A  (hn = A*x + D)
            A = st.tile([P, 4], F32)
            D = st.tile([P, 4], F32)
            nc.vector.tensor_tensor(out=A, in0=rstd, in1=gcol_sb, op=ALU.mult)
            nc.vector.tensor_tensor(out=D, in0=mq[:, 0:4], in1=A, op=ALU.mult)
            nc.vector.tensor_tensor(out=D, in0=bcol_sb, in1=D, op=ALU.subtract)
            # x is NEVER normalized: hn = A*x + D is folded instead —
            #   scores: S^T = x^T (A*q2) + const(i); const cancels in softmax
            #   vT:     Wv' = A*WvT (rows), bias row += D @ WvT
            D_bf = st.tile([P, 4], BF16)
            nc.vector.tensor_copy(out=D_bf, in_=D)
            pbv = psmm.tile([1, 512], F32, tag="mm", name="pbv")
            for kb in range(KB):
                nc.tensor.matmul(pbv, D_bf[:, kb:kb + 1], wv_sb[:, kb, :],
                                 start=(kb == 0), stop=(kb == KB - 1))
            nc.vector.tensor_copy(out=rpad[0:1, :], in_=pbv)
            pbvb = psmm.tile([P, 512], F32, tag="mm", name="pbvb")
            nc.tensor.matmul(pbvb, e0, rpad, start=True, stop=True)
            nc.vector.tensor_tensor(out=bv_sb, in0=bv_sb, in1=pbvb, op=ALU.add)
            for kb in range(KB):
                nc.vector.tensor_scalar_mul(wv_sb[:, kb, :], wv_sb[:, kb, :],
                                            A[:, kb:kb + 1])
            # ---- stage B: vT projection straight off RAW x (A/D folded)
            for nt in range(NT):
                # vT for this chunk's 4 j-tiles
                for j4 in range(4):
                    jt = nt * 4 + j4
                    pv = psmm.tile([P, 512], F32, tag="mm", name="pv")
                    for kb in range(KB):
                        nc.tensor.matmul(
                            pv, x_sb[:, kb, jt * P:(jt + 1) * P], wv_sb[:, kb, :],
                            start=(kb == 0), stop=(kb == KB - 1))
                    nc.vector.tensor_tensor(out=vT[:, jt, :], in0=pv, in1=bv_sb,
                                            op=ALU.add)

            # hq: off the stats -> first-projection critical path
            Aq = st.tile([P, 4], F32)
            Dq = st.tile([P, 4], F32)
            nc.vector.tensor_scalar_mul(Aq, A, SCALE)
            nc.vector.tensor_scalar_mul(Dq, D, SCALE)
            for kb in range(KB):
                nc.vector.tensor_scalar(
                    out=hq[:, kb, :], in0=xq_sb[:, kb, :],
                    scalar1=Aq[:, kb:kb + 1], scalar2=Dq[:, kb:kb + 1],
                    op0=ALU.mult, op1=ALU.add)
            # fold the out-proj bias into the residual: xq_sb += bo (per chan)
            for kb in range(KB):
                nc.vector.tensor_scalar_add(
                    xq_sb[:, kb, :], xq_sb[:, kb, :], boc_sb[:, kb:kb + 1])
            # q = (Wq @ hq_scaled) + bq*SCALE  (SCALE folded into hq/bqc),
            # then q2 = Wk^T @ q: folds the k projection through the score
            # matmul (S^T = (Wk hn)^T q = hn^T (Wk^T q)); bk's contribution
            # is constant along the softmax axis and cancels exactly.
            # i2-half-major order so q2 of half 0 overlaps q of half 1.
            for i2 in range(IH):
                for blk in range(KB):
                    pq = psmm.tile([P, 512], F32, tag="mm", name="pq")
                    for kb in range(KB):
                        nc.tensor.matmul(
                            pq, wq_sb[:, kb, blk * P:(blk + 1) * P],
                            hq[:, kb, i2 * 512:(i2 + 1) * 512],
                            start=(kb == 0), stop=(kb == KB - 1))
                    nc.vector.tensor_scalar_add(
                        q_sb[:, blk, i2 * 512:(i2 + 1) * 512], pq,
                        bqc_sb[:, blk:blk + 1])
                for blk in range(KB):
                    p2 = psmm.tile([P, 512], F32, tag="mm", name="p2")
                    for kb in range(KB):
                        nc.tensor.matmul(
                            p2, wk_sb[:, kb, blk * P:(blk + 1) * P],
                            q_sb[:, kb, i2 * 512:(i2 + 1) * 512],
                            start=(kb == 0), stop=(kb == KB - 1))
                    nc.vector.tensor_scalar_mul(
                        q2_sb[:, blk, i2 * 512:(i2 + 1) * 512], p2,
                        A[:, blk:blk + 1])

            # ---- stage C: attention, pipelined over (ih, jt) ----
            # Per step: scores+exp for (ih, jt); PV matmuls for the previous
            # step; DVE row-sum accumulation into sden_sb. The ih=0 epilogue
            # is emitted a few steps into ih=1 so PE never waits on the
            # reciprocal/broadcast chain.
            pv_ps = {}
            sden_sb = {}
            ets = {}

            def epilogue(ih):
                # Copy unnormalized PV to SBUF right away (frees the pv psum
                # accumulators for the next half) and run the Wo projection on
                # it; the softmax 1/s is folded in afterwards:
                #   out = (Wo @ PV)/s + bo + xq
                on = ep.tile([P, KB, 512], BF16, tag="on", name=f"on{ih}")
                for cc in range(KB):
                    nc.vector.tensor_copy(out=on[:, cc, :], in_=pv_ps[ih][cc])
                # cross-partition sum of sden_sb -> s[1, 512]; r = 1/s
                sden = pvp.tile([1, 512], F32, tag="sden", name=f"sden{ih}")
                nc.tensor.matmul(sden, ones_f, sden_sb[ih], start=True, stop=True)
                nc.vector.reciprocal(out=rpad[0:1, :], in_=sden)
                rb = psmm.tile([P, 512], F32, tag="mm", name=f"rb{ih}")
                nc.tensor.matmul(rb, e0, rpad, start=True, stop=True)
                rbs = ep.tile([P, 512], F32, tag="rbs", name=f"rbs{ih}")
                nc.vector.tensor_copy(out=rbs, in_=rb)
                for blk in range(KB):
                    po = psmm.tile([P, 512], F32, tag="mm", name=f"po{ih}")
                    for cc in range(KB):
                        nc.tensor.matmul(
                            po, wo_sb[:, cc, blk * P:(blk + 1) * P], on[:, cc, :],
                            start=(cc == 0), stop=(cc == KB - 1))
                    ot = epo.tile([P, 512], F32, tag="ot", name=f"ot{ih}")
                    nc.vector.tensor_tensor(out=ot, in0=po, in1=rbs, op=ALU.mult)
                    nc.vector.tensor_tensor(
                        out=ot, in0=ot,
                        in1=xq_sb[:, blk, ih * 512:(ih + 1) * 512], op=ALU.add)
                    nc.sync.dma_start(
                        out=out_r[:, blk, ih * 512:(ih + 1) * 512], in_=ot)

            NSTEP = IH * JT
            for step in range(NSTEP + 1):
                if step < NSTEP:
                    ih, jt = divmod(step, JT)
                    if jt == 0:
                        pv_ps[ih] = [pvp.tile([P, 512], F32, tag=f"pv{cc}",
                                              name=f"pv{ih}_{cc}")
                                     for cc in range(KB)]
                        sden_sb[ih] = ep.tile([P, 512], F32, tag="sd",
                                              name=f"sd{ih}")
                    ss = psmm.tile([P, 512], F32, tag="mm", name="ss")
                    for kb in range(KB):
                        nc.tensor.matmul(
                            ss, x_sb[:, kb, jt * P:(jt + 1) * P],
                            q2_sb[:, kb, ih * 512:(ih + 1) * 512],
                            start=(kb == 0), stop=(kb == KB - 1))
                    et = etp.tile([P, 512], BF16, tag="et", name="et")
                    nc.scalar.activation(out=et, in_=ss, func=AF.Exp)
                    ets[step] = et
                    if jt == 0:
                        nc.vector.tensor_copy(out=sden_sb[ih], in_=et)
                    else:
                        nc.vector.tensor_tensor(out=sden_sb[ih], in0=sden_sb[ih],
                                                in1=et, op=ALU.add)
                if step >= 1:
                    pih, pjt = divmod(step - 1, JT)
                    et = ets.pop(step - 1)
                    for cc in range(KB):
                        nc.tensor.matmul(
                            pv_ps[pih][cc], vT[:, pjt, cc * P:(cc + 1) * P],
                            et, start=(pjt == 0), stop=(pjt == JT - 1))
                if step == JT + 2:
                    epilogue(0)
            epilogue(1)

    nc.finalize()
    return nc


_NC = None


def _get_nc():
    global _NC
    if _NC is None:
        _NC = build_nc()
    return _NC


def _col(v):
    """[C] f32 -> [P, KB] with c = blk*128 + p."""
    return np.ascontiguousarray(np.asarray(v, np.float32).reshape(KB, P).T)


def _make_in_maps(inputs):
    x = np.asarray(inputs["x"], np.float32).reshape(2, C, N)
    x_bf = x.astype(ml_dtypes.bfloat16)
    wqT = np.ascontiguousarray(np.asarray(inputs["Wq"], np.float32).T).astype(ml_dtypes.bfloat16)
    wkR = np.ascontiguousarray(np.asarray(inputs["Wk"], np.float32)).astype(ml_dtypes.bfloat16)
    wvT = np.ascontiguousarray(np.asarray(inputs["Wv"], np.float32).T).astype(ml_dtypes.bfloat16)
    woT = np.ascontiguousarray(np.asarray(inputs["Wo"], np.float32).T).astype(ml_dtypes.bfloat16)
    gcol = _col(inputs["gamma"])
    bcol = _col(inputs["beta"])
    bqc = _col(np.asarray(inputs["bq"], np.float32) * SCALE)
    boc = _col(inputs["bo"])
    bvv = np.ascontiguousarray(np.asarray(inputs["bv"], np.float32))

    pidx = np.arange(P)
    gavg = np.where(pidx[:, None] // 16 == pidx[None, :] // 16,
                    np.float32(1.0 / 16.0), np.float32(0.0))

    common = dict(wq=wqT, wk=wkR, wv=wvT, wo=woT, gcol=gcol, bcol=bcol,
                  bqc=bqc, boc=boc, bv=bvv, gavg=gavg)
    in_maps = []
    for core in range(8):
        b, qc = core // 4, core % 4
        in_maps.append(dict(
            common,
            xb=np.ascontiguousarray(x_bf[b]),
            xq=np.ascontiguousarray(x[b][:, qc * NQ:(qc + 1) * NQ]),
        ))
    return in_maps


def run(inputs, trace=False):
    nc = _get_nc()
    in_maps = _make_in_maps(inputs)
    res = run_bass_kernel_spmd(nc, in_maps, core_ids=list(range(8)), trace=trace)
    y = np.empty((2, C, N), np.float32)
    for core in range(8):
        b, qc = core // 4, core % 4
        y[b][:, qc * NQ:(qc + 1) * NQ] = res.results[core]["out"]
    return y.reshape(2, C, 64, 64), res


def kernel(**inputs):
    y, _ = run(inputs, trace=False)
    return y


# revision 35
# speedup vs baseline: 1.1981x; 1.0917x over previous
"""AttnBlock (GroupNorm + single-head attention over HW + residual) on 8 trn2 cores.

Sharding: core = b*4 + qc  (b in 0..1 batch, qc in 0..3 query-column chunk).
Each core receives the full batch element x[b] ([512, 4096], pre-cast bf16)
plus its query chunk x[b][:, qc*1024:(qc+1)*1024] (f32), computes groupnorm +
k/v over all tokens (redundantly per batch) and attention/out-proj for its
1024 query rows.

Layout conventions (everything keyed off channel c = blk*128 + p):
  x/hn, k, q SBUF tiles: [p=128, blk=4, tokens]   (c on partitions)
  vT: [p=128 (token within j-tile), jt=32, c=512] (tokens on partitions)
Attention is computed transposed (S^T[j, i]) so that no on-chip transpose is
ever required: S^T = k(c,j-tile)^T x q(c,i); softmax row-sums accumulate on
DVE over j-tiles and are reduced across partitions with one ones-matmul; the
1/s row is broadcast to 128 partitions with one padded matmul.
"""

import numpy as np
import ml_dtypes

import concourse.bass as bass
import concourse.bacc as bacc
import concourse.mybir as mybir
import concourse.tile as tile
from concourse.bass_utils import run_bass_kernel_spmd

P = 128
C = 512
N = 4096          # tokens per batch element (H*W)
NQ = 1024         # query tokens per core
KB = C // P       # 4 channel blocks
NT = N // 512     # 8 token tiles of 512
JT = N // P       # 32 j tiles of 128
IH = NQ // 512    # 2 query halves of 512
EPS = 1e-6
SCALE = float(C) ** -0.5

F32 = mybir.dt.float32
BF16 = mybir.dt.bfloat16
AF = mybir.ActivationFunctionType
ALU = mybir.AluOpType


def build_nc():
    nc = bacc.Bacc()

    xb = nc.dram_tensor("xb", [C, N], BF16, kind="ExternalInput")
    xq = nc.dram_tensor("xq", [C, NQ], F32, kind="ExternalInput")
    wq = nc.dram_tensor("wq", [C, C], BF16, kind="ExternalInput")  # [cin, cout]
    wk = nc.dram_tensor("wk", [C, C], BF16, kind="ExternalInput")  # RAW Wk
    wv = nc.dram_tensor("wv", [C, C], BF16, kind="ExternalInput")
    wo = nc.dram_tensor("wo", [C, C], BF16, kind="ExternalInput")
    gcol = nc.dram_tensor("gcol", [P, KB], F32, kind="ExternalInput")   # gamma
    bcol = nc.dram_tensor("bcol", [P, KB], F32, kind="ExternalInput")   # beta
    bqc = nc.dram_tensor("bqc", [P, KB], F32, kind="ExternalInput")     # bq*SCALE
    boc = nc.dram_tensor("boc", [P, KB], F32, kind="ExternalInput")     # bo
    bvc = nc.dram_tensor("bvc", [P, KB], F32, kind="ExternalInput")
    gavg = nc.dram_tensor("gavg", [P, P], F32, kind="ExternalInput")
    out = nc.dram_tensor("out", [C, NQ], F32, kind="ExternalOutput")

    xb_r = xb[:].rearrange("(blk p) n -> p blk n", p=P)
    xq_r = xq[:].rearrange("(blk p) n -> p blk n", p=P)
    out_r = out[:].rearrange("(blk p) n -> p blk n", p=P)
    wq_r = wq[:].rearrange("(kb p) co -> p kb co", p=P)
    wk_r = wk[:].rearrange("(kb p) co -> p kb co", p=P)
    wv_r = wv[:].rearrange("(kb p) co -> p kb co", p=P)
    wo_r = wo[:].rearrange("(kb p) co -> p kb co", p=P)

    with tile.TileContext(nc) as tc:
        with (
            tc.tile_pool(name="big", bufs=1) as big,
            tc.tile_pool(name="st", bufs=1) as st,
            tc.tile_pool(name="et", bufs=8) as etp,
            tc.tile_pool(name="ep", bufs=2) as ep,
            tc.tile_pool(name="epo", bufs=4) as epo,
            tc.tile_pool(name="mm", bufs=3, space="PSUM") as psmm,
            tc.tile_pool(name="pvp", bufs=1, space="PSUM") as pvp,
        ):
            # ---- persistent tiles ----
            x_sb = big.tile([P, KB, N], BF16)    # x, normalized in place -> hn
            hq = big.tile([P, KB, NQ], BF16)
            xT = big.tile([P, JT, C], BF16)   # raw x, transposed: [j, cin]
            q_sb = big.tile([P, KB, NQ], BF16)
            q2_sb = big.tile([P, KB, NQ], BF16)
            xq_sb = big.tile([P, KB, NQ], F32)
            wq_sb = big.tile([P, KB, C], BF16)
            wk_sb = big.tile([P, KB, C], BF16)
            wv_sb = big.tile([P, KB, C], BF16)
            wo_sb = big.tile([P, KB, C], BF16)
            gcol_sb = big.tile([P, KB], F32)
            bcol_sb = big.tile([P, KB], F32)
            bqc_sb = big.tile([P, KB], F32)
            boc_sb = big.tile([P, KB], F32)
            bvc_sb = big.tile([P, KB], F32)
            gavg_sb = big.tile([P, P], F32)
            ones_f = big.tile([P, 1], F32)
            e0 = big.tile([P, P], F32)      # row 0 = 1, else 0 (for row bcast)
            rpad = big.tile([P, 512], F32)  # row 0 = 1/s, else 0

            # x streams in first so bn_stats can start ASAP
            XPIECES = [(nt * 512, 512) for nt in range(NT)]
            for off, w in XPIECES:
                nc.sync.dma_start(out=x_sb[:, :, off:off + w],
                                  in_=xb_r[:, :, off:off + w])
            nc.sync.dma_start(out=wq_sb, in_=wq_r)
            nc.sync.dma_start(out=wk_sb, in_=wk_r)
            nc.sync.dma_start(out=wv_sb, in_=wv_r)
            nc.sync.dma_start(out=wo_sb, in_=wo_r)
            nc.sync.dma_start(out=gcol_sb, in_=gcol[:])
            nc.sync.dma_start(out=bcol_sb, in_=bcol[:])
            nc.sync.dma_start(out=bqc_sb, in_=bqc[:])
            nc.sync.dma_start(out=boc_sb, in_=boc[:])
            nc.sync.dma_start(out=gavg_sb, in_=gavg[:])
            nc.sync.dma_start(out=bvc_sb, in_=bvc[:])
            nc.vector.memset(ones_f, 1.0)
            nc.vector.memset(e0, 0.0)
            nc.vector.memset(e0[0:1, :], 1.0)
            nc.vector.memset(rpad, 0.0)
            nc.sync.dma_start(out=xq_sb, in_=xq_r)
            # transposed copy of raw x for the PV contraction (XBAR path,
            # off the PE); not needed until stage C so it rides idle DMA time
            for jt in range(JT):
                nc.sync.dma_start_transpose(
                    out=xT[:, jt, :], in_=xb[:][:, jt * P:(jt + 1) * P])

            # ---- stage A: groupnorm stats ----
            # Split across engines: channel-block 0 of each chunk goes to ACT
            # (Copy/Square passes with accum_out -> per-partition sums),
            # blocks 1..3 go to DVE bn_stats. Both finish ~5us sooner than
            # DVE-alone. ACT passes are grouped by func to avoid table loads.
            NPC = len(XPIECES)
            stats = st.tile([P, KB - 1, NPC, 6], F32)
            adump = st.tile([P, 512], F32)
            accs = st.tile([P, NPC], F32)
            accq = st.tile([P, NPC], F32)
            for i, (off, w) in enumerate(XPIECES):
                nc.scalar.activation(out=adump[:, :w], in_=x_sb[:, 0, off:off + w],
                                     func=AF.Copy, accum_out=accs[:, i:i + 1])
                nc.scalar.activation(out=adump[:, :w], in_=x_sb[:, 0, off:off + w],
                                     func=AF.Square, accum_out=accq[:, i:i + 1])
                for kb in range(1, KB):
                    nc.vector.bn_stats(out=stats[:, kb - 1, i, :],
                                       in_=x_sb[:, kb, off:off + w])
            mv = st.tile([P, KB - 1, 2], F32)
            for kb in range(1, KB):
                nc.vector.bn_aggr(out=mv[:, kb - 1, :], in_=stats[:, kb - 1, :, :])

            # per-partition stats -> per-group stats -> per-channel A/D
            stat8 = st.tile([P, 8], F32)
            sm0 = st.tile([P, 2], F32)
            nc.vector.reduce_sum(out=sm0[:, 0:1], in_=accs,
                                 axis=mybir.AxisListType.X)
            nc.vector.reduce_sum(out=sm0[:, 1:2], in_=accq,
                                 axis=mybir.AxisListType.X)
            nc.vector.tensor_scalar_mul(sm0, sm0, 1.0 / float(N))
            nc.vector.tensor_copy(out=stat8[:, 0:1], in_=sm0[:, 0:1])
            nc.vector.tensor_copy(out=stat8[:, 4:5], in_=sm0[:, 1:2])
            nc.vector.tensor_copy(out=stat8[:, 1:4], in_=mv[:, :, 0])
            nc.vector.tensor_tensor(out=stat8[:, 5:8], in0=mv[:, :, 0],
                                    in1=mv[:, :, 0], op=ALU.mult)
            nc.vector.tensor_tensor(out=stat8[:, 5:8], in0=stat8[:, 5:8],
                                    in1=mv[:, :, 1], op=ALU.add)
            psb = psmm.tile([P, 8], F32, tag="mm", name="psb")
            nc.tensor.matmul(psb, gavg_sb, stat8, start=True, stop=True)
            mq = st.tile([P, 8], F32)
            nc.vector.tensor_copy(out=mq, in_=psb)
            varg = st.tile([P, 4], F32)
            nc.vector.tensor_tensor(out=varg, in0=mq[:, 0:4], in1=mq[:, 0:4],
                                    op=ALU.mult)
            nc.vector.tensor_tensor(out=varg, in0=mq[:, 4:8], in1=varg,
                                    op=ALU.subtract)
            rstd = st.tile([P, 4], F32)
            eps_sb = st.tile([P, 1], F32)
            nc.vector.memset(eps_sb, EPS)
            nc.scalar.activation(out=rstd, in_=varg, func=AF.Sqrt, bias=eps_sb)
            nc.vector.reciprocal(out=rstd, in_=rstd)
            expdump = st.tile([P, 1], F32)
            nc.scalar.activation(out=expdump, in_=eps_sb, func=AF.Exp)
            # A = rstd*gamma ; D = beta - mean*A  (hn = A*x + D)
            A = st.tile([P, 4], F32)
            D = st.tile([P, 4], F32)
            nc.vector.tensor_tensor(out=A, in0=rstd, in1=gcol_sb, op=ALU.mult)
            nc.vector.tensor_tensor(out=D, in0=mq[:, 0:4], in1=A, op=ALU.mult)
            nc.vector.tensor_tensor(out=D, in0=bcol_sb, in1=D, op=ALU.subtract)
            # x is NEVER normalized: hn = A*x + D is folded instead —
            #   scores: S^T = x^T (A*q2) + const(i); const cancels in softmax
            #   vT:     Wv' = A*WvT (rows), bias row += D @ WvT
            # hq: off the stats -> first-projection critical path
            Aq = st.tile([P, 4], F32)
            Dq = st.tile([P, 4], F32)
            nc.vector.tensor_scalar_mul(Aq, A, SCALE)
            nc.vector.tensor_scalar_mul(Dq, D, SCALE)
            for kb in range(KB):
                nc.vector.tensor_scalar(
                    out=hq[:, kb, :], in0=xq_sb[:, kb, :],
                    scalar1=Aq[:, kb:kb + 1], scalar2=Dq[:, kb:kb + 1],
                    op0=ALU.mult, op1=ALU.add)
            # q = (Wq @ hq_scaled) + bq*SCALE  (SCALE folded into hq/bqc),
            # then q2 = Wk^T @ q: folds the k projection through the score
            # matmul (S^T = (Wk hn)^T q = hn^T (Wk^T q)); bk's contribution
            # is constant along the softmax axis and cancels exactly.
            # i2-half-major order so q2 of half 0 overlaps q of half 1.
            for i2 in range(IH):
                for blk in range(KB):
                    pq = psmm.tile([P, 512], F32, tag="mm", name="pq")
                    for kb in range(KB):
                        nc.tensor.matmul(
                            pq, wq_sb[:, kb, blk * P:(blk + 1) * P],
                            hq[:, kb, i2 * 512:(i2 + 1) * 512],
                            start=(kb == 0), stop=(kb == KB - 1))
                    nc.vector.tensor_scalar_add(
                        q_sb[:, blk, i2 * 512:(i2 + 1) * 512], pq,
                        bqc_sb[:, blk:blk + 1])
                for blk in range(KB):
                    p2 = psmm.tile([P, 512], F32, tag="mm", name="p2")
                    for kb in range(KB):
                        nc.tensor.matmul(
                            p2, wk_sb[:, kb, blk * P:(blk + 1) * P],
                            q_sb[:, kb, i2 * 512:(i2 + 1) * 512],
                            start=(kb == 0), stop=(kb == KB - 1))
                    nc.vector.tensor_scalar_mul(
                        q2_sb[:, blk, i2 * 512:(i2 + 1) * 512], p2,
                        A[:, blk:blk + 1])

            # bvd[cout] = D @ WvT + bv (column form), then wobvd = Wo @ bvd
            # folds into the residual: o = Wo(Wv' PVx)/s + Wo bvd + bo + xq
            D_bf = st.tile([P, 4], BF16)
            nc.vector.tensor_copy(out=D_bf, in_=D)
            bvdc = st.tile([P, KB], F32)
            for co in range(KB):
                pbc = psmm.tile([P, 1], F32, tag="mm", name="pbc")
                for kb in range(KB):
                    nc.tensor.matmul(pbc, wv_sb[:, kb, co * P:(co + 1) * P],
                                     D_bf[:, kb:kb + 1],
                                     start=(kb == 0), stop=(kb == KB - 1))
                nc.vector.tensor_copy(out=bvdc[:, co:co + 1], in_=pbc)
            nc.vector.tensor_tensor(out=bvdc, in0=bvdc, in1=bvc_sb, op=ALU.add)
            bvdc_bf = st.tile([P, KB], BF16)
            nc.vector.tensor_copy(out=bvdc_bf, in_=bvdc)
            wobvd = st.tile([P, KB], F32)
            for blk in range(KB):
                pwb = psmm.tile([P, 1], F32, tag="mm", name="pwb")
                for co in range(KB):
                    nc.tensor.matmul(pwb, wo_sb[:, co, blk * P:(blk + 1) * P],
                                     bvdc_bf[:, co:co + 1],
                                     start=(co == 0), stop=(co == KB - 1))
                nc.vector.tensor_copy(out=wobvd[:, blk:blk + 1], in_=pwb)
            for kb in range(KB):
                nc.vector.tensor_scalar_mul(wv_sb[:, kb, :], wv_sb[:, kb, :],
                                            A[:, kb:kb + 1])
            # fold the out-proj bias AND Wo@bvd into the residual
            for kb in range(KB):
                nc.vector.tensor_scalar_add(
                    xq_sb[:, kb, :], xq_sb[:, kb, :], boc_sb[:, kb:kb + 1])
            for kb in range(KB):
                nc.vector.tensor_scalar_add(
                    xq_sb[:, kb, :], xq_sb[:, kb, :], wobvd[:, kb:kb + 1])
            # ---- stage C: attention, pipelined over (ih, jt) ----
            # Per step: scores+exp for (ih, jt); PV matmuls for the previous
            # step; DVE row-sum accumulation into sden_sb. The ih=0 epilogue
            # is emitted a few steps into ih=1 so PE never waits on the
            # reciprocal/broadcast chain.
            pv_ps = {}
            sden_sb = {}
            ets = {}

            def epilogue(ih):
                # Copy unnormalized PV to SBUF right away (frees the pv psum
                # accumulators for the next half) and run the Wo projection on
                # it; the softmax 1/s is folded in afterwards:
                #   out = (Wo @ PV)/s + bo + xq
                on = ep.tile([P, KB, 512], BF16, tag="on", name=f"on{ih}")
                for cc in range(KB):
                    nc.vector.tensor_copy(out=on[:, cc, :], in_=pv_ps[ih][cc])
                # v-side weights applied here: on2 = Wv' @ PVx
                on2 = ep.tile([P, KB, 512], BF16, tag="on2", name=f"on2{ih}")
                for co in range(KB):
                    pw = psmm.tile([P, 512], F32, tag="mm", name=f"pw{ih}")
                    for cc in range(KB):
                        nc.tensor.matmul(
                            pw, wv_sb[:, cc, co * P:(co + 1) * P], on[:, cc, :],
                            start=(cc == 0), stop=(cc == KB - 1))
                    nc.vector.tensor_copy(out=on2[:, co, :], in_=pw)
                # cross-partition sum of sden_sb -> s[1, 512]; r = 1/s
                sden = pvp.tile([1, 512], F32, tag="sden", name=f"sden{ih}")
                nc.tensor.matmul(sden, ones_f, sden_sb[ih], start=True, stop=True)
                nc.vector.reciprocal(out=rpad[0:1, :], in_=sden)
                rb = psmm.tile([P, 512], F32, tag="mm", name=f"rb{ih}")
                nc.tensor.matmul(rb, e0, rpad, start=True, stop=True)
                rbs = ep.tile([P, 512], F32, tag="rbs", name=f"rbs{ih}")
                nc.vector.tensor_copy(out=rbs, in_=rb)
                for blk in range(KB):
                    po = psmm.tile([P, 512], F32, tag="mm", name=f"po{ih}")
                    for cc in range(KB):
                        nc.tensor.matmul(
                            po, wo_sb[:, cc, blk * P:(blk + 1) * P], on2[:, cc, :],
                            start=(cc == 0), stop=(cc == KB - 1))
                    ot = epo.tile([P, 512], F32, tag="ot", name=f"ot{ih}")
                    nc.vector.tensor_tensor(out=ot, in0=po, in1=rbs, op=ALU.mult)
                    nc.vector.tensor_tensor(
                        out=ot, in0=ot,
                        in1=xq_sb[:, blk, ih * 512:(ih + 1) * 512], op=ALU.add)
                    nc.sync.dma_start(
                        out=out_r[:, blk, ih * 512:(ih + 1) * 512], in_=ot)

            NSTEP = IH * JT
            for step in range(NSTEP + 1):
                if step < NSTEP:
                    ih, jt = divmod(step, JT)
                    if jt == 0:
                        pv_ps[ih] = [pvp.tile([P, 512], F32, tag=f"pv{cc}",
                                              name=f"pv{ih}_{cc}")
                                     for cc in range(KB)]
                        sden_sb[ih] = ep.tile([P, 512], F32, tag="sd",
                                              name=f"sd{ih}")
                    ss = psmm.tile([P, 512], F32, tag="mm", name="ss")
                    for kb in range(KB):
                        nc.tensor.matmul(
                            ss, x_sb[:, kb, jt * P:(jt + 1) * P],
                            q2_sb[:, kb, ih * 512:(ih + 1) * 512],
                            start=(kb == 0), stop=(kb == KB - 1))
                    et = etp.tile([P, 512], BF16, tag="et", name="et")
                    nc.scalar.activation(out=et, in_=ss, func=AF.Exp)
                    ets[step] = et
                    if jt == 0:
                        nc.vector.tensor_copy(out=sden_sb[ih], in_=et)
                    else:
                        nc.vector.tensor_tensor(out=sden_sb[ih], in0=sden_sb[ih],
                                                in1=et, op=ALU.add)
                if step >= 1:
                    pih, pjt = divmod(step - 1, JT)
                    et = ets.pop(step - 1)
                    for cc in range(KB):
                        nc.tensor.matmul(
                            pv_ps[pih][cc], xT[:, pjt, cc * P:(cc + 1) * P],
                            et, start=(pjt == 0), stop=(pjt == JT - 1))
                if step == JT + 2:
                    epilogue(0)
            epilogue(1)

    nc.finalize()
    return nc


_NC = None


def _get_nc():
    global _NC
    if _NC is None:
        _NC = build_nc()
    return _NC


def _col(v):
    """[C] f32 -> [P, KB] with c = blk*128 + p."""
    return np.ascontiguousarray(np.asarray(v, np.float32).reshape(KB, P).T)


def _make_in_maps(inputs):
    x = np.asarray(inputs["x"], np.float32).reshape(2, C, N)
    x_bf = x.astype(ml_dtypes.bfloat16)
    wqT = np.ascontiguousarray(np.asarray(inputs["Wq"], np.float32).T).astype(ml_dtypes.bfloat16)
    wkR = np.ascontiguousarray(np.asarray(inputs["Wk"], np.float32)).astype(ml_dtypes.bfloat16)
    wvT = np.ascontiguousarray(np.asarray(inputs["Wv"], np.float32).T).astype(ml_dtypes.bfloat16)
    woT = np.ascontiguousarray(np.asarray(inputs["Wo"], np.float32).T).astype(ml_dtypes.bfloat16)
    gcol = _col(inputs["gamma"])
    bcol = _col(inputs["beta"])
    bqc = _col(np.asarray(inputs["bq"], np.float32) * SCALE)
    boc = _col(inputs["bo"])
    bvcc = _col(inputs["bv"])

    pidx = np.arange(P)
    gavg = np.where(pidx[:, None] // 16 == pidx[None, :] // 16,
                    np.float32(1.0 / 16.0), np.float32(0.0))

    common = dict(wq=wqT, wk=wkR, wv=wvT, wo=woT, gcol=gcol, bcol=bcol,
                  bqc=bqc, boc=boc, bvc=bvcc, gavg=gavg)
    in_maps = []
    for core in range(8):
        b, qc = core // 4, core % 4
        in_maps.append(dict(
            common,
            xb=np.ascontiguousarray(x_bf[b]),
            xq=np.ascontiguousarray(x[b][:, qc * NQ:(qc + 1) * NQ]),
        ))
    return in_maps


def run(inputs, trace=False):
    nc = _get_nc()
    in_maps = _make_in_maps(inputs)
    res = run_bass_kernel_spmd(nc, in_maps, core_ids=list(range(8)), trace=trace)
    y = np.empty((2, C, N), np.float32)
    for core in range(8):
        b, qc = core // 4, core % 4
        y[b][:, qc * NQ:(qc + 1) * NQ] = res.results[core]["out"]
    return y.reshape(2, C, 64, 64), res


def kernel(**inputs):
    y, _ = run(inputs, trace=False)
    return y


# revision 37
# speedup vs baseline: 1.2201x; 1.0184x over previous
"""AttnBlock (GroupNorm + single-head attention over HW + residual) on 8 trn2 cores.

Sharding: core = b*4 + qc  (b in 0..1 batch, qc in 0..3 query-column chunk).
Each core receives the full batch element x[b] ([512, 4096], pre-cast bf16)
plus its query chunk x[b][:, qc*1024:(qc+1)*1024] (f32), computes groupnorm +
k/v over all tokens (redundantly per batch) and attention/out-proj for its
1024 query rows.

Layout conventions (everything keyed off channel c = blk*128 + p):
  x/hn, k, q SBUF tiles: [p=128, blk=4, tokens]   (c on partitions)
  vT: [p=128 (token within j-tile), jt=32, c=512] (tokens on partitions)
Attention is computed transposed (S^T[j, i]) so that no on-chip transpose is
ever required: S^T = k(c,j-tile)^T x q(c,i); softmax row-sums accumulate on
DVE over j-tiles and are reduced across partitions with one ones-matmul; the
1/s row is broadcast to 128 partitions with one padded matmul.
"""

import numpy as np
import ml_dtypes

import concourse.bass as bass
import concourse.bacc as bacc
import concourse.mybir as mybir
import concourse.tile as tile
from concourse.bass_utils import run_bass_kernel_spmd

P = 128
C = 512
N = 4096          # tokens per batch element (H*W)
NQ = 1024         # query tokens per core
KB = C // P       # 4 channel blocks
NT = N // 512     # 8 token tiles of 512
JT = N // P       # 32 j tiles of 128
IH = NQ // 512    # 2 query halves of 512
EPS = 1e-6
SCALE = float(C) ** -0.5

F32 = mybir.dt.float32
BF16 = mybir.dt.bfloat16
AF = mybir.ActivationFunctionType
ALU = mybir.AluOpType


def build_nc():
    nc = bacc.Bacc()

    xb = nc.dram_tensor("xb", [C, N], BF16, kind="ExternalInput")
    xq = nc.dram_tensor("xq", [C, NQ], F32, kind="ExternalInput")
    xqb = nc.dram_tensor("xqb", [C, NQ], BF16, kind="ExternalInput")
    wq = nc.dram_tensor("wq", [C, C], BF16, kind="ExternalInput")  # [cin, cout]
    wk = nc.dram_tensor("wk", [C, C], BF16, kind="ExternalInput")  # RAW Wk
    wv = nc.dram_tensor("wv", [C, C], BF16, kind="ExternalInput")
    wo = nc.dram_tensor("wo", [C, C], BF16, kind="ExternalInput")
    gcol = nc.dram_tensor("gcol", [P, KB], F32, kind="ExternalInput")   # gamma
    bcol = nc.dram_tensor("bcol", [P, KB], F32, kind="ExternalInput")   # beta
    bqc = nc.dram_tensor("bqc", [P, KB], F32, kind="ExternalInput")     # bq*SCALE
    boc = nc.dram_tensor("boc", [P, KB], F32, kind="ExternalInput")     # bo
    bvc = nc.dram_tensor("bvc", [P, KB], F32, kind="ExternalInput")
    gavg = nc.dram_tensor("gavg", [P, P], F32, kind="ExternalInput")
    out = nc.dram_tensor("out", [C, NQ], F32, kind="ExternalOutput")

    xb_r = xb[:].rearrange("(blk p) n -> p blk n", p=P)
    xq_r = xq[:].rearrange("(blk p) n -> p blk n", p=P)
    xqb_r = xqb[:].rearrange("(blk p) n -> p blk n", p=P)
    out_r = out[:].rearrange("(blk p) n -> p blk n", p=P)
    wq_r = wq[:].rearrange("(kb p) co -> p kb co", p=P)
    wk_r = wk[:].rearrange("(kb p) co -> p kb co", p=P)
    wv_r = wv[:].rearrange("(kb p) co -> p kb co", p=P)
    wo_r = wo[:].rearrange("(kb p) co -> p kb co", p=P)

    with tile.TileContext(nc) as tc:
        with (
            tc.tile_pool(name="big", bufs=1) as big,
            tc.tile_pool(name="st", bufs=1) as st,
            tc.tile_pool(name="et", bufs=8) as etp,
            tc.tile_pool(name="ep", bufs=2) as ep,
            tc.tile_pool(name="epo", bufs=4) as epo,
            tc.tile_pool(name="mm", bufs=3, space="PSUM") as psmm,
            tc.tile_pool(name="pvp", bufs=1, space="PSUM") as pvp,
        ):
            # ---- persistent tiles ----
            x_sb = big.tile([P, KB, N], BF16)    # x, normalized in place -> hn
            hq = big.tile([P, KB, NQ], BF16)
            xT = big.tile([P, JT, C], BF16)   # raw x, transposed: [j, cin]
            q_sb = big.tile([P, KB, NQ], BF16)
            q2_sb = big.tile([P, KB, NQ], BF16)
            xq_sb = big.tile([P, KB, NQ], F32)
            xqb_sb = big.tile([P, KB, NQ], BF16)
            wq_sb = big.tile([P, KB, C], BF16)
            wk_sb = big.tile([P, KB, C], BF16)
            wv_sb = big.tile([P, KB, C], BF16)
            wo_sb = big.tile([P, KB, C], BF16)
            gcol_sb = big.tile([P, KB], F32)
            bcol_sb = big.tile([P, KB], F32)
            bqc_sb = big.tile([P, KB], F32)
            boc_sb = big.tile([P, KB], F32)
            bvc_sb = big.tile([P, KB], F32)
            gavg_sb = big.tile([P, P], F32)
            ones_f = big.tile([P, 1], F32)
            e0 = big.tile([P, P], F32)      # row 0 = 1, else 0 (for row bcast)
            rpad = big.tile([P, 512], F32)  # row 0 = 1/s, else 0

            # x streams in first so bn_stats can start ASAP
            XPIECES = [(0, 256), (256, 256)] + [
                (nt * 512, 512) for nt in range(1, NT)]
            for off, w in XPIECES:
                nc.sync.dma_start(out=x_sb[:, :, off:off + w],
                                  in_=xb_r[:, :, off:off + w])
            nc.sync.dma_start(out=xqb_sb, in_=xqb_r)
            nc.sync.dma_start(out=wq_sb, in_=wq_r)
            nc.sync.dma_start(out=wk_sb, in_=wk_r)
            nc.sync.dma_start(out=wv_sb, in_=wv_r)
            nc.sync.dma_start(out=wo_sb, in_=wo_r)
            nc.sync.dma_start(out=gcol_sb, in_=gcol[:])
            nc.sync.dma_start(out=bcol_sb, in_=bcol[:])
            nc.sync.dma_start(out=bqc_sb, in_=bqc[:])
            nc.sync.dma_start(out=boc_sb, in_=boc[:])
            nc.sync.dma_start(out=gavg_sb, in_=gavg[:])
            nc.sync.dma_start(out=bvc_sb, in_=bvc[:])
            nc.vector.memset(ones_f, 1.0)
            nc.vector.memset(e0, 0.0)
            nc.vector.memset(e0[0:1, :], 1.0)
            nc.vector.memset(rpad, 0.0)
            nc.sync.dma_start(out=xq_sb, in_=xq_r)
            # transposed copy of raw x for the PV contraction (XBAR path,
            # off the PE); not needed until stage C so it rides idle DMA time
            for jt in range(JT):
                nc.sync.dma_start_transpose(
                    out=xT[:, jt, :], in_=xb[:][:, jt * P:(jt + 1) * P])

            # ---- stage A: groupnorm stats ----
            # Split across engines: channel-block 0 of each chunk goes to ACT
            # (Copy/Square passes with accum_out -> per-partition sums),
            # blocks 1..3 go to DVE bn_stats. Both finish ~5us sooner than
            # DVE-alone. ACT passes are grouped by func to avoid table loads.
            NPC = len(XPIECES)
            stats = st.tile([P, KB - 1, NPC, 6], F32)
            adump = st.tile([P, 512], F32)
            accs = st.tile([P, NPC], F32)
            accq = st.tile([P, NPC], F32)
            for i, (off, w) in enumerate(XPIECES):
                nc.scalar.activation(out=adump[:, :w], in_=x_sb[:, 0, off:off + w],
                                     func=AF.Copy, accum_out=accs[:, i:i + 1])
                nc.scalar.activation(out=adump[:, :w], in_=x_sb[:, 0, off:off + w],
                                     func=AF.Square, accum_out=accq[:, i:i + 1])
                for kb in range(1, KB):
                    nc.vector.bn_stats(out=stats[:, kb - 1, i, :],
                                       in_=x_sb[:, kb, off:off + w])
            mv = st.tile([P, KB - 1, 2], F32)
            for kb in range(1, KB):
                nc.vector.bn_aggr(out=mv[:, kb - 1, :], in_=stats[:, kb - 1, :, :])

            # per-partition stats -> per-group stats -> per-channel A/D
            stat8 = st.tile([P, 8], F32)
            sm0 = st.tile([P, 2], F32)
            nc.vector.reduce_sum(out=sm0[:, 0:1], in_=accs,
                                 axis=mybir.AxisListType.X)
            nc.vector.reduce_sum(out=sm0[:, 1:2], in_=accq,
                                 axis=mybir.AxisListType.X)
            nc.vector.tensor_scalar_mul(sm0, sm0, 1.0 / float(N))
            nc.vector.tensor_copy(out=stat8[:, 0:1], in_=sm0[:, 0:1])
            nc.vector.tensor_copy(out=stat8[:, 4:5], in_=sm0[:, 1:2])
            nc.vector.tensor_copy(out=stat8[:, 1:4], in_=mv[:, :, 0])
            nc.vector.tensor_tensor(out=stat8[:, 5:8], in0=mv[:, :, 0],
                                    in1=mv[:, :, 0], op=ALU.mult)
            nc.vector.tensor_tensor(out=stat8[:, 5:8], in0=stat8[:, 5:8],
                                    in1=mv[:, :, 1], op=ALU.add)
            psb = psmm.tile([P, 8], F32, tag="mm", name="psb")
            nc.tensor.matmul(psb, gavg_sb, stat8, start=True, stop=True)
            mq = st.tile([P, 8], F32)
            nc.vector.tensor_copy(out=mq, in_=psb)
            varg = st.tile([P, 4], F32)
            nc.vector.tensor_tensor(out=varg, in0=mq[:, 0:4], in1=mq[:, 0:4],
                                    op=ALU.mult)
            nc.vector.tensor_tensor(out=varg, in0=mq[:, 4:8], in1=varg,
                                    op=ALU.subtract)
            rstd = st.tile([P, 4], F32)
            eps_sb = st.tile([P, 1], F32)
            nc.vector.memset(eps_sb, EPS)
            nc.scalar.activation(out=rstd, in_=varg, func=AF.Sqrt, bias=eps_sb)
            nc.vector.reciprocal(out=rstd, in_=rstd)
            expdump = st.tile([P, 1], F32)
            nc.scalar.activation(out=expdump, in_=eps_sb, func=AF.Exp)
            # A = rstd*gamma ; D = beta - mean*A  (hn = A*x + D)
            A = st.tile([P, 4], F32)
            D = st.tile([P, 4], F32)
            nc.vector.tensor_tensor(out=A, in0=rstd, in1=gcol_sb, op=ALU.mult)
            nc.vector.tensor_tensor(out=D, in0=mq[:, 0:4], in1=A, op=ALU.mult)
            nc.vector.tensor_tensor(out=D, in0=bcol_sb, in1=D, op=ALU.subtract)
            # x is NEVER normalized: hn = A*x + D is folded instead —
            #   scores: S^T = x^T (A*q2) + const(i); const cancels in softmax
            #   vT:     Wv' = A*WvT (rows), bias row += D @ WvT
            # hq: off the stats -> first-projection critical path
            Aq = st.tile([P, 4], F32)
            Dq = st.tile([P, 4], F32)
            nc.vector.tensor_scalar_mul(Aq, A, SCALE)
            nc.vector.tensor_scalar_mul(Dq, D, SCALE)
            for kb in range(KB):
                nc.vector.tensor_scalar(
                    out=hq[:, kb, :], in0=xqb_sb[:, kb, :],
                    scalar1=Aq[:, kb:kb + 1], scalar2=Dq[:, kb:kb + 1],
                    op0=ALU.mult, op1=ALU.add)
            # q = (Wq @ hq_scaled) + bq*SCALE  (SCALE folded into hq/bqc),
            # then q2 = Wk^T @ q: folds the k projection through the score
            # matmul (S^T = (Wk hn)^T q = hn^T (Wk^T q)); bk's contribution
            # is constant along the softmax axis and cancels exactly.
            # i2-half-major order so q2 of half 0 overlaps q of half 1.
            for i2 in range(IH):
                for blk in range(KB):
                    pq = psmm.tile([P, 512], F32, tag="mm", name="pq")
                    for kb in range(KB):
                        nc.tensor.matmul(
                            pq, wq_sb[:, kb, blk * P:(blk + 1) * P],
                            hq[:, kb, i2 * 512:(i2 + 1) * 512],
                            start=(kb == 0), stop=(kb == KB - 1))
                    nc.vector.tensor_scalar_add(
                        q_sb[:, blk, i2 * 512:(i2 + 1) * 512], pq,
                        bqc_sb[:, blk:blk + 1])
                for blk in range(KB):
                    p2 = psmm.tile([P, 512], F32, tag="mm", name="p2")
                    for kb in range(KB):
                        nc.tensor.matmul(
                            p2, wk_sb[:, kb, blk * P:(blk + 1) * P],
                            q_sb[:, kb, i2 * 512:(i2 + 1) * 512],
                            start=(kb == 0), stop=(kb == KB - 1))
                    nc.vector.tensor_scalar_mul(
                        q2_sb[:, blk, i2 * 512:(i2 + 1) * 512], p2,
                        A[:, blk:blk + 1])

            # bvd[cout] = D @ WvT + bv (column form), then wobvd = Wo @ bvd
            # folds into the residual: o = Wo(Wv' PVx)/s + Wo bvd + bo + xq
            D_bf = st.tile([P, 4], BF16)
            nc.vector.tensor_copy(out=D_bf, in_=D)
            bvdc = st.tile([P, KB], F32)
            for co in range(KB):
                pbc = psmm.tile([P, 1], F32, tag="mm", name="pbc")
                for kb in range(KB):
                    nc.tensor.matmul(pbc, wv_sb[:, kb, co * P:(co + 1) * P],
                                     D_bf[:, kb:kb + 1],
                                     start=(kb == 0), stop=(kb == KB - 1))
                nc.vector.tensor_copy(out=bvdc[:, co:co + 1], in_=pbc)
            nc.vector.tensor_tensor(out=bvdc, in0=bvdc, in1=bvc_sb, op=ALU.add)
            bvdc_bf = st.tile([P, KB], BF16)
            nc.vector.tensor_copy(out=bvdc_bf, in_=bvdc)
            wobvd = st.tile([P, KB], F32)
            for blk in range(KB):
                pwb = psmm.tile([P, 1], F32, tag="mm", name="pwb")
                for co in range(KB):
                    nc.tensor.matmul(pwb, wo_sb[:, co, blk * P:(blk + 1) * P],
                                     bvdc_bf[:, co:co + 1],
                                     start=(co == 0), stop=(co == KB - 1))
                nc.vector.tensor_copy(out=wobvd[:, blk:blk + 1], in_=pwb)
            for kb in range(KB):
                nc.vector.tensor_scalar_mul(wv_sb[:, kb, :], wv_sb[:, kb, :],
                                            A[:, kb:kb + 1])
            # fold the out-proj bias AND Wo@bvd into the residual
            for kb in range(KB):
                nc.vector.tensor_scalar_add(
                    xq_sb[:, kb, :], xq_sb[:, kb, :], boc_sb[:, kb:kb + 1])
            for kb in range(KB):
                nc.vector.tensor_scalar_add(
                    xq_sb[:, kb, :], xq_sb[:, kb, :], wobvd[:, kb:kb + 1])
            # ---- stage C: attention, pipelined over (ih, jt) ----
            # Per step: scores+exp for (ih, jt); PV matmuls for the previous
            # step; DVE row-sum accumulation into sden_sb. The ih=0 epilogue
            # is emitted a few steps into ih=1 so PE never waits on the
            # reciprocal/broadcast chain.
            pv_ps = {}
            sden_sb = {}
            ets = {}

            def epilogue(ih):
                # Copy unnormalized PV to SBUF right away (frees the pv psum
                # accumulators for the next half) and run the Wo projection on
                # it; the softmax 1/s is folded in afterwards:
                #   out = (Wo @ PV)/s + bo + xq
                on = ep.tile([P, KB, 512], BF16, tag="on", name=f"on{ih}")
                for cc in range(KB):
                    nc.vector.tensor_copy(out=on[:, cc, :], in_=pv_ps[ih][cc])
                # v-side weights applied here: on2 = Wv' @ PVx
                on2 = ep.tile([P, KB, 512], BF16, tag="on2", name=f"on2{ih}")
                for co in range(KB):
                    pw = psmm.tile([P, 512], F32, tag="mm", name=f"pw{ih}")
                    for cc in range(KB):
                        nc.tensor.matmul(
                            pw, wv_sb[:, cc, co * P:(co + 1) * P], on[:, cc, :],
                            start=(cc == 0), stop=(cc == KB - 1))
                    nc.vector.tensor_copy(out=on2[:, co, :], in_=pw)
                # cross-partition sum of sden_sb -> s[1, 512]; r = 1/s
                sden = pvp.tile([1, 512], F32, tag="sden", name=f"sden{ih}")
                nc.tensor.matmul(sden, ones_f, sden_sb[ih], start=True, stop=True)
                nc.vector.reciprocal(out=rpad[0:1, :], in_=sden)
                rb = psmm.tile([P, 512], F32, tag="mm", name=f"rb{ih}")
                nc.tensor.matmul(rb, e0, rpad, start=True, stop=True)
                rbs = ep.tile([P, 512], F32, tag="rbs", name=f"rbs{ih}")
                nc.vector.tensor_copy(out=rbs, in_=rb)
                for blk in range(KB):
                    po = psmm.tile([P, 512], F32, tag="mm", name=f"po{ih}")
                    for cc in range(KB):
                        nc.tensor.matmul(
                            po, wo_sb[:, cc, blk * P:(blk + 1) * P], on2[:, cc, :],
                            start=(cc == 0), stop=(cc == KB - 1))
                    ot = epo.tile([P, 512], F32, tag="ot", name=f"ot{ih}")
                    nc.vector.tensor_tensor(out=ot, in0=po, in1=rbs, op=ALU.mult)
                    nc.vector.tensor_tensor(
                        out=ot, in0=ot,
                        in1=xq_sb[:, blk, ih * 512:(ih + 1) * 512], op=ALU.add)
                    nc.sync.dma_start(
                        out=out_r[:, blk, ih * 512:(ih + 1) * 512], in_=ot)

            NSTEP = IH * JT
            for step in range(NSTEP + 1):
                if step < NSTEP:
                    ih, jt = divmod(step, JT)
                    if jt == 0:
                        pv_ps[ih] = [pvp.tile([P, 512], F32, tag=f"pv{cc}",
                                              name=f"pv{ih}_{cc}")
                                     for cc in range(KB)]
                        sden_sb[ih] = ep.tile([P, 512], F32, tag="sd",
                                              name=f"sd{ih}")
                    ss = psmm.tile([P, 512], F32, tag="mm", name="ss")
                    for kb in range(KB):
                        nc.tensor.matmul(
                            ss, x_sb[:, kb, jt * P:(jt + 1) * P],
                            q2_sb[:, kb, ih * 512:(ih + 1) * 512],
                            start=(kb == 0), stop=(kb == KB - 1))
                    et = etp.tile([P, 512], BF16, tag="et", name="et")
                    nc.scalar.activation(out=et, in_=ss, func=AF.Exp)
                    ets[step] = et
                    if jt == 0:
                        nc.vector.tensor_copy(out=sden_sb[ih], in_=et)
                    else:
                        nc.vector.tensor_tensor(out=sden_sb[ih], in0=sden_sb[ih],
                                                in1=et, op=ALU.add)
                if step >= 1:
                    pih, pjt = divmod(step - 1, JT)
                    et = ets.pop(step - 1)
                    for cc in range(KB):
                        nc.tensor.matmul(
                            pv_ps[pih][cc], xT[:, pjt, cc * P:(cc + 1) * P],
                            et, start=(pjt == 0), stop=(pjt == JT - 1))
                if step == JT + 2:
                    epilogue(0)
            epilogue(1)

    nc.finalize()
    return nc


_NC = None


def _get_nc():
    global _NC
    if _NC is None:
        _NC = build_nc()
    return _NC


def _col(v):
    """[C] f32 -> [P, KB] with c = blk*128 + p."""
    return np.ascontiguousarray(np.asarray(v, np.float32).reshape(KB, P).T)


def _make_in_maps(inputs):
    x = np.asarray(inputs["x"], np.float32).reshape(2, C, N)
    x_bf = x.astype(ml_dtypes.bfloat16)
    wqT = np.ascontiguousarray(np.asarray(inputs["Wq"], np.float32).T).astype(ml_dtypes.bfloat16)
    wkR = np.ascontiguousarray(np.asarray(inputs["Wk"], np.float32)).astype(ml_dtypes.bfloat16)
    wvT = np.ascontiguousarray(np.asarray(inputs["Wv"], np.float32).T).astype(ml_dtypes.bfloat16)
    woT = np.ascontiguousarray(np.asarray(inputs["Wo"], np.float32).T).astype(ml_dtypes.bfloat16)
    gcol = _col(inputs["gamma"])
    bcol = _col(inputs["beta"])
    bqc = _col(np.asarray(inputs["bq"], np.float32) * SCALE)
    boc = _col(inputs["bo"])
    bvcc = _col(inputs["bv"])

    pidx = np.arange(P)
    gavg = np.where(pidx[:, None] // 16 == pidx[None, :] // 16,
                    np.float32(1.0 / 16.0), np.float32(0.0))

    common = dict(wq=wqT, wk=wkR, wv=wvT, wo=woT, gcol=gcol, bcol=bcol,
                  bqc=bqc, boc=boc, bvc=bvcc, gavg=gavg)
    in_maps = []
    for core in range(8):
        b, qc = core // 4, core % 4
        in_maps.append(dict(
            common,
            xb=np.ascontiguousarray(x_bf[b]),
            xq=np.ascontiguousarray(x[b][:, qc * NQ:(qc + 1) * NQ]),
            xqb=np.ascontiguousarray(x_bf[b][:, qc * NQ:(qc + 1) * NQ]),
        ))
    return in_maps


def run(inputs, trace=False):
    nc = _get_nc()
    in_maps = _make_in_maps(inputs)
    res = run_bass_kernel_spmd(nc, in_maps, core_ids=list(range(8)), trace=trace)
    y = np.empty((2, C, N), np.float32)
    for core in range(8):
        b, qc = core // 4, core % 4
        y[b][:, qc * NQ:(qc + 1) * NQ] = res.results[core]["out"]
    return y.reshape(2, C, 64, 64), res


def kernel(**inputs):
    y, _ = run(inputs, trace=False)
    return y
